# revision 1
# baseline (speedup 1.0000x reference)
"""Trainium2 Bass kernel for retrieval-knn attention classifier (nn_MA_51866025067137).

Strategy (8 NeuronCores):
  Phase 1 — memory_keys sharded along N (12800 keys/core, padded 100000->102400
  with dummy rows).  Each core computes cosine-similarity ranking values for all
  256 queries against its shard (fp32r matmuls on the PE; keys pre-normalized so
  the matmul directly yields cosine ranking values) and extracts its local
  top-32 (value, index) per query with DVE max8/max_index/match_replace, using a
  low-9-mantissa-bit packing trick to recover in-segment indices.
  Host — merges the 8x40 candidates per row, re-scores them exactly in fp32,
  and gathers the global top-32 key vectors.
  Phase 2 — batch sharded (32 queries/core): memory-attention module
  (tanh(qWq + knnWm + b) -> scores -> softmax -> weighted sum) and classifier,
  via small fp32r matmuls; the softmax-weighted sum is a block-diagonal matmul.
"""

import numpy as np

import concourse.bacc as bacc
import concourse.mybir as mybir
from concourse.tile import TileContext
from concourse.bass_utils import run_bass_kernel_spmd
from concourse.masks import make_identity

# problem dims (hardcoded per harness contract)
B, N, D = 256, 100000, 512
A, C, K = 256, 100, 32
NC_CORES = 8
NPAD = 102400             # 8 * 12800
SHARD = NPAD // NC_CORES  # 12800
CHUNK = 512               # keys per inner loop step
NCHUNK = SHARD // CHUNK   # 25
SEG = 512                 # max8 segment width (9-bit in-segment index)
NSEG = SHARD // SEG       # 25
L1W = NSEG * 8            # 200
BROWS = B // NC_CORES     # 32 rows per core in phase 2
KLOC = 40                 # local candidates per core per row
CAND = NC_CORES * KLOC    # 320 merged candidates per row

f32 = mybir.dt.float32
f32r = mybir.dt.float32r
u32 = mybir.dt.uint32

_PH1 = None
_PH2 = None


def _build_phase1():
    nc = bacc.Bacc("TRN2", target_bir_lowering=False)
    khatT = nc.dram_tensor("khatT", [NCHUNK, 128, 4 * CHUNK], f32r, kind="ExternalInput")
    qT = nc.dram_tensor("qT", [D, B], f32r, kind="ExternalInput")
    win_out = nc.dram_tensor("win", [B, KLOC], f32, kind="ExternalOutput")
    pos_out = nc.dram_tensor("pos", [B, KLOC], u32, kind="ExternalOutput")

    with TileContext(nc) as tc:
        with (
            tc.tile_pool(name="const", bufs=1) as constp,
            tc.tile_pool(name="qpool", bufs=1) as qpool,
            tc.tile_pool(name="keys", bufs=6) as keyp,
            tc.tile_pool(name="packed", bufs=8) as packp,
            tc.tile_pool(name="l1", bufs=1) as l1p,
            tc.tile_pool(name="small", bufs=1) as smallp,
            tc.tile_pool(name="psum", bufs=2, space="PSUM") as psump,
        ):
            # constants: AND-mask (0xFFFFFE00) per partition; iota 0..511
            mask_t = constp.tile([128, 1], u32, tag="mask")
            nc.vector.memset(mask_t[:], 0xFFFFFE00)
            iota_t = constp.tile([128, CHUNK], u32, tag="iota")
            nc.gpsimd.iota(iota_t[:], pattern=[[1, CHUNK]], base=0,
                           channel_multiplier=0)

            # load qT and relu in place
            qTr = []
            for dc in range(4):
                t = qpool.tile([128, B], f32r, tag=f"qt{dc}")
                nc.sync.dma_start(out=t[:], in_=qT[dc * 128:(dc + 1) * 128, :])
                nc.scalar.activation(t[:], t[:], mybir.ActivationFunctionType.Relu)
                qTr.append(t)

            L1 = [l1p.tile([128, L1W], f32, tag=f"l1_{qt}", name=f"l1_{qt}") for qt in range(2)]

            for c in range(NCHUNK):
                kt = keyp.tile([128, 4 * CHUNK], f32r, tag="kt")
                nc.sync.dma_start(out=kt[:], in_=khatT[c, :, :])
                for qt in range(2):
                    ps = psump.tile([128, CHUNK], f32, tag=f"sim{qt}")
                    for dc in range(4):
                        nc.tensor.matmul(
                            ps[:],
                            lhsT=qTr[dc][:, qt * 128:(qt + 1) * 128],
                            rhs=kt[:, dc * CHUNK:(dc + 1) * CHUNK],
                            start=(dc == 0), stop=(dc == 3),
                        )
                    # evict (ACT), pack on GPSIMD: packed = (sim & mask) | iota
                    ev = packp.tile([128, CHUNK], f32, tag=f"ev{qt}")
                    nc.scalar.copy(out=ev[:], in_=ps[:])
                    pk = packp.tile([128, CHUNK], f32, tag=f"pk{qt}")
                    nc.vector.scalar_tensor_tensor(
                        out=pk[:].bitcast(u32), in0=ev[:].bitcast(u32),
                        scalar=mask_t[:], in1=iota_t[:],
                        op0=mybir.AluOpType.bitwise_and,
                        op1=mybir.AluOpType.bitwise_or,
                    )
                    nc.vector.max(out=L1[qt][:, c * 8:(c + 1) * 8], in_=pk[:])

            # extraction: 5 rounds of top-8 from L1 (400 wide)
            for qt in range(2):
                win = smallp.tile([128, KLOC], f32, tag=f"win{qt}")
                pos = smallp.tile([128, KLOC], u32, tag=f"pos{qt}")
                for r in range(5):
                    w8 = win[:, r * 8:(r + 1) * 8]
                    nc.vector.max(out=w8, in_=L1[qt][:])
                    nc.vector.max_index(out=pos[:, r * 8:(r + 1) * 8],
                                        in_max=w8, in_values=L1[qt][:])
                    if r < 4:
                        nc.vector.match_replace(out=L1[qt][:], in_to_replace=w8,
                                                in_values=L1[qt][:],
                                                imm_value=-3.0e38)
                nc.sync.dma_start(out=win_out[qt * 128:(qt + 1) * 128, :], in_=win[:])
                nc.sync.dma_start(out=pos_out[qt * 128:(qt + 1) * 128, :], in_=pos[:])
    nc.finalize()
    return nc


def _build_phase2():
    nc = bacc.Bacc("TRN2", target_bir_lowering=False)
    qT_in = nc.dram_tensor("qT", [D, BROWS], f32r, kind="ExternalInput")       # pre-relu
    knn_in = nc.dram_tensor("knn", [BROWS * K, D], f32r, kind="ExternalInput")
    knnT_in = nc.dram_tensor("knnT", [D, BROWS * K], f32r, kind="ExternalInput")
    Wq_in = nc.dram_tensor("Wq", [D, A], f32r, kind="ExternalInput")
    Wm_in = nc.dram_tensor("Wm", [D, A], f32r, kind="ExternalInput")
    Ws_in = nc.dram_tensor("Ws", [A, 1], f32r, kind="ExternalInput")
    bqm_in = nc.dram_tensor("bqm", [A, 1], f32, kind="ExternalInput")          # bq+bm
    Wc_in = nc.dram_tensor("Wc", [2 * D, C], f32r, kind="ExternalInput")
    out_d = nc.dram_tensor("out", [BROWS, C], f32, kind="ExternalOutput")      # +bc host
    escratch = nc.dram_tensor("escratch", [1, BROWS * K], f32)                 # bounce

    NCD = BROWS * K  # 1024

    with TileContext(nc) as tc:
        with (
            tc.tile_pool(name="big", bufs=1) as bigp,
            tc.tile_pool(name="small", bufs=1) as smallp,
            tc.tile_pool(name="psum", bufs=1, space="PSUM") as psump,
        ):
            # ---- load inputs (M-padded tiles to satisfy fp32r col_grp=0xf) ----
            qT = [smallp.tile([128, 128], f32r, tag=f"qT{dc}", name=f"qTt{dc}") for dc in range(4)]
            for dc in range(4):
                nc.vector.memset(qT[dc][:].bitcast(u32), 0)
                nc.sync.dma_start(out=qT[dc][:, :BROWS],
                                  in_=qT_in[dc * 128:(dc + 1) * 128, :])
                nc.scalar.activation(qT[dc][:, :BROWS], qT[dc][:, :BROWS],
                                     mybir.ActivationFunctionType.Relu)
            knnall = bigp.tile([128, 8 * D], f32r, tag="knnall")
            nc.sync.dma_start(out=knnall[:].rearrange("p (t d) -> p t d", t=8),
                              in_=knn_in[:].rearrange("(t p) d -> p t d", p=128))
            knn = [knnall[:, t * D:(t + 1) * D] for t in range(8)]
            knnTall = bigp.tile([128, 4 * NCD], f32r, tag="knnTall")
            nc.sync.dma_start(out=knnTall[:].rearrange("p (dc c) -> p dc c", dc=4),
                              in_=knnT_in[:].rearrange("(dc p) c -> p dc c", p=128))
            knnT = [knnTall[:, dc * NCD:(dc + 1) * NCD] for dc in range(4)]
            Wqall = smallp.tile([128, 4 * A], f32r, tag="Wqall")
            nc.sync.dma_start(out=Wqall[:].rearrange("p (dc a) -> p dc a", dc=4),
                              in_=Wq_in[:].rearrange("(dc p) a -> p dc a", p=128))
            Wq = [Wqall[:, dc * A:(dc + 1) * A] for dc in range(4)]
            Wmall = smallp.tile([128, 4 * A], f32r, tag="Wmall")
            nc.sync.dma_start(out=Wmall[:].rearrange("p (dc a) -> p dc a", dc=4),
                              in_=Wm_in[:].rearrange("(dc p) a -> p dc a", p=128))
            Wm = [Wmall[:, dc * A:(dc + 1) * A] for dc in range(4)]
            Ws = [smallp.tile([128, 128], f32r, tag=f"Ws{at}", name=f"Wst{at}") for at in range(2)]
            bqm = [smallp.tile([128, 1], f32, tag=f"bqm{at}", name=f"bqmt{at}") for at in range(2)]
            for at in range(2):
                nc.vector.memset(Ws[at][:].bitcast(u32), 0)
                nc.sync.dma_start(out=Ws[at][:, :1],
                                  in_=Ws_in[at * 128:(at + 1) * 128, :])
                nc.sync.dma_start(out=bqm[at][:],
                                  in_=bqm_in[at * 128:(at + 1) * 128, :])
            Wcall = smallp.tile([128, 8 * C], f32r, tag="Wcall")
            nc.sync.dma_start(out=Wcall[:].rearrange("p (m j) -> p m j", m=8),
                              in_=Wc_in[:].rearrange("(m p) j -> p m j", p=128))
            Wc = [Wcall[:, m * C:(m + 1) * C] for m in range(8)]
            ones = smallp.tile([128, 2], f32r, tag="ones")
            nc.vector.memset(ones[:].bitcast(u32), 0)
            nc.vector.memset(ones[:, :1].bitcast(u32), 0x3F800000)
            # mask4[p, j] = 1.0 iff j == p // 32
            mask4 = smallp.tile([128, 4], f32, tag="mask4")
            nc.vector.memset(mask4[:], 1.0)
            nc.gpsimd.affine_select(out=mask4[:], in_=mask4[:],
                                    compare_op=mybir.AluOpType.is_ge, fill=0.0,
                                    base=0, pattern=[[-32, 4]], channel_multiplier=1)
            nc.gpsimd.affine_select(out=mask4[:], in_=mask4[:],
                                    compare_op=mybir.AluOpType.is_ge, fill=0.0,
                                    base=31, pattern=[[32, 4]], channel_multiplier=-1)
            ident = smallp.tile([128, 128], f32, tag="ident")
            make_identity(nc, ident[:])

            # ---- qprojT [2][128a, 32] ----
            qprojT = [smallp.tile([128, BROWS], f32, tag=f"qp{at}", name=f"qpt{at}") for at in range(2)]
            for at in range(2):
                ps = psump.tile([128, BROWS], f32, tag="ps_a")
                for dc in range(4):
                    nc.tensor.matmul(
                        ps[:],
                        lhsT=Wq[dc][:, at * 128:(at + 1) * 128],
                        rhs=qT[dc][:, :BROWS],
                        start=(dc == 0), stop=(dc == 3))
                nc.scalar.copy(out=qprojT[at][:], in_=ps[:])

            # ---- hT = tanh(kprojT + qprojT_bcast + bqm) ; scores ----
            sc_ps = psump.tile([128, NCD], f32, tag="ps_sc")
            for at in range(2):
                kp = psump.tile([128, NCD], f32, tag="ps_kp", bufs=2)
                for dc in range(4):
                    for half in range(2):
                        nc.tensor.matmul(
                            kp[:, half * 512:(half + 1) * 512],
                            lhsT=Wm[dc][:, at * 128:(at + 1) * 128],
                            rhs=knnT[dc][:, half * 512:(half + 1) * 512],
                            start=(dc == 0), stop=(dc == 3))
                hT = bigp.tile([128, NCD], f32r, tag=f"hT{at}")
                qb = qprojT[at][:, :, None].to_broadcast([128, BROWS, K])
                nc.vector.tensor_tensor(
                    hT[:].rearrange("p (q k) -> p q k", k=K),
                    kp[:].rearrange("p (q k) -> p q k", k=K),
                    qb, mybir.AluOpType.add)
                nc.scalar.activation(hT[:], hT[:], mybir.ActivationFunctionType.Tanh,
                                     bias=bqm[at][:])
                for half in range(2):
                    nc.tensor.matmul(
                        sc_ps[:, half * 512:(half + 1) * 512],
                        lhsT=Ws[at][:],
                        rhs=hT[:, half * 512:(half + 1) * 512],
                        start=(at == 0), stop=(at == 1))
            e_row = smallp.tile([1, NCD], f32, tag="e_row")
            nc.scalar.activation(e_row[:], sc_ps[:1, :],
                                 mybir.ActivationFunctionType.Exp)
            # bounce through DRAM to redistribute [1, 1024] -> [128, 8]
            nc.sync.dma_start(out=escratch[:, :], in_=e_row[:, :])
            e_col = smallp.tile([128, 8], f32, tag="e_col")
            nc.sync.dma_start(out=e_col[:],
                              in_=escratch[0, :].rearrange("(t p) -> p t", p=128))

            # ---- block-diag softmax weights (M-padded), den, attended ----
            w2 = [bigp.tile([128, 128], f32r, tag=f"w2_{t}", name=f"w2t{t}") for t in range(8)]
            for t in range(8):
                nc.vector.memset(w2[t][:].bitcast(u32), 0)
                nc.vector.tensor_scalar_mul(w2[t][:, 4 * t:4 * t + 4], mask4[:],
                                            e_col[:, t:t + 1])
            den_ps = psump.tile([128, 2], f32, tag="ps_a")
            for t in range(8):
                nc.tensor.matmul(den_ps[:], lhsT=w2[t][:], rhs=ones[:],
                                 start=(t == 0), stop=(t == 7))
            att_ps = psump.tile([128, D], f32, tag="ps_kp", bufs=2)
            for t in range(8):
                nc.tensor.matmul(att_ps[:], lhsT=w2[t][:], rhs=knn[t],
                                 start=(t == 0), stop=(t == 7))
            rden = smallp.tile([BROWS, 1], f32, tag="rden")
            nc.vector.reciprocal(rden[:], den_ps[:BROWS, :1])
            att = smallp.tile([BROWS, D], f32, tag="att_sb")
            nc.vector.tensor_scalar_mul(att[:], att_ps[:BROWS, :], rden[:])

            # ---- attendedT via PE transpose (plain fp32) ----
            attT = [smallp.tile([128, 128], f32r, tag=f"attT{dc}", name=f"attTt{dc}") for dc in range(4)]
            for dc in range(4):
                tp = psump.tile([128, BROWS], f32, tag="ps_a")
                nc.tensor.transpose(tp[:], att[:, dc * 128:(dc + 1) * 128],
                                    ident[:BROWS, :BROWS])
                nc.vector.memset(attT[dc][:].bitcast(u32), 0)
                nc.scalar.copy(out=attT[dc][:, :BROWS], in_=tp[:])

            # ---- classifier ----
            out_ps = psump.tile([128, C], f32, tag="ps_out")
            for m in range(8):
                lhsT = qT[m] if m < 4 else attT[m - 4]
                nc.tensor.matmul(out_ps[:], lhsT=lhsT[:], rhs=Wc[m],
                                 start=(m == 0), stop=(m == 7))
            out_sb = smallp.tile([BROWS, C], f32, tag="out_sb")
            nc.scalar.copy(out=out_sb[:], in_=out_ps[:BROWS, :])
            nc.sync.dma_start(out=out_d[:, :], in_=out_sb[:])
    nc.finalize()
    return nc


def _phase1_nc():
    global _PH1
    if _PH1 is None:
        _PH1 = _build_phase1()
    return _PH1


def _phase2_nc():
    global _PH2
    if _PH2 is None:
        _PH2 = _build_phase2()
    return _PH2


def kernel(query_feat, memory_keys, Wq, bq, Wm, bm, Ws, bs, Wc, bc):
    query_feat = np.asarray(query_feat, np.float32)
    memory_keys = np.asarray(memory_keys, np.float32)

    # ---- host prep: pad + normalize + transpose + shard keys ----
    kn = np.sqrt((memory_keys ** 2).sum(axis=1))
    khat = memory_keys * (1.0 / kn)[:, None]
    pad = np.full((NPAD - N, D), -1.0 / np.sqrt(D), np.float32)
    khat_pad = np.concatenate([khat.astype(np.float32), pad], axis=0)
    qT_full = np.ascontiguousarray(query_feat.T)  # [512, 256]

    ph1 = _phase1_nc()
    in_maps = []
    for c in range(NC_CORES):
        sh = khat_pad[c * SHARD:(c + 1) * SHARD]          # [12800, 512]
        arr = np.ascontiguousarray(
            sh.reshape(NCHUNK, CHUNK, 4, 128).transpose(0, 3, 2, 1)
        ).reshape(NCHUNK, 128, 4 * CHUNK)
        in_maps.append({"khatT": arr, "qT": qT_full})
    res1 = run_bass_kernel_spmd(ph1, in_maps, core_ids=list(range(NC_CORES)))

    # ---- host merge: recover indices, exact re-score of candidates ----
    all_gidx = np.zeros((B, NC_CORES, KLOC), np.int64)
    for c in range(NC_CORES):
        win = res1.results[c]["win"].view(np.uint32)
        pos = res1.results[c]["pos"].astype(np.int64)   # 0..399 in L1
        seg = pos // 8
        within = (win & np.uint32(0x1FF)).astype(np.int64)
        all_gidx[:, c, :] = seg * SEG + within + c * SHARD
    gidx = all_gidx.reshape(B, CAND)
    safe = np.minimum(gidx, N - 1)
    q32 = np.maximum(query_feat, 0)
    cand_keys = memory_keys[safe]                       # [256, 320, 512]
    dots = np.einsum("bd,bcd->bc", q32, cand_keys, optimize=True)
    cos = dots / np.maximum(
        np.linalg.norm(q32, axis=1)[:, None] * kn[safe], np.float32(1e-8))
    cos[gidx >= N] = -np.inf                            # mask dummy-pad hits
    order = np.argsort(-cos, axis=1, kind="stable")[:, :K]
    top_idx = np.take_along_axis(safe, order, axis=1)   # [256, 32]
    knn = memory_keys[top_idx]                          # [256, 32, 512]

    # ---- phase 2 (batch sharded) ----
    ph2 = _phase2_nc()
    bqm = (np.asarray(bq, np.float32) + np.asarray(bm, np.float32)).reshape(A, 1)
    Wq_a = np.ascontiguousarray(np.asarray(Wq, np.float32))
    Wm_a = np.ascontiguousarray(np.asarray(Wm, np.float32))
    Ws_a = np.ascontiguousarray(np.asarray(Ws, np.float32))
    Wc_a = np.ascontiguousarray(np.asarray(Wc, np.float32))
    in_maps2 = []
    for c in range(NC_CORES):
        rows = slice(c * BROWS, (c + 1) * BROWS)
        knn_c = knn[rows].reshape(BROWS * K, D)
        in_maps2.append({
            "qT": np.ascontiguousarray(query_feat[rows].T),
            "knn": np.ascontiguousarray(knn_c),
            "knnT": np.ascontiguousarray(knn_c.T),
            "Wq": Wq_a, "Wm": Wm_a, "Ws": Ws_a, "bqm": bqm, "Wc": Wc_a,
        })
    res2 = run_bass_kernel_spmd(ph2, in_maps2, core_ids=list(range(NC_CORES)))
    out = np.concatenate([res2.results[c]["out"] for c in range(NC_CORES)], axis=0)
    return (out + np.asarray(bc, np.float32)[None, :]).astype(np.float32)



# revision 8
# speedup vs baseline: 2.1445x; 2.1445x over previous
"""Trainium2 Bass kernel for retrieval-knn attention classifier (nn_MA_51866025067137).

Strategy (8 NeuronCores):
  Phase 1 — memory_keys sharded along N (12800 keys/core, padded 100000->102400
  with zero rows).  Keys and queries are L2-normalized on host and cast to
  fp8-e4m3 (x64 scale).  Each core ranks all 256 queries against its shard with
  DoubleRow fp8 matmuls (2 k-tiles per matmul, 256-deep contraction), then the
  DVE folds each chunk-pair with three levels of pairwise tensor_max
  (f32 PSUM -> bf16, then bf16 at 2x) yielding one bf16 max per 8-key window.
  The [128, 1600] window-max arrays are DMA'd out; the host picks the top-8
  windows per 4096-key supergroup, expands each window to its 8 keys,
  re-scores candidates exactly in fp32, and takes the global top-32.
  Phase 2 — batch sharded (32 queries/core): memory-attention module
  (tanh(qWq + knnWm + b) -> scores -> softmax -> weighted sum) and classifier
  in bf16.  Scores are computed candidate-major so softmax needs no transpose
  bounce; the weighted sum and classifier are small PE matmuls.
"""

import numpy as np
import ml_dtypes

import concourse.bacc as bacc
import concourse.mybir as mybir
from concourse.tile import TileContext
from concourse.bass_utils import run_bass_kernel_spmd

# problem dims (hardcoded per harness contract)
B, N, D = 256, 100000, 512
A, C, K = 256, 100, 32
NC_CORES = 8
NPAD = 102400             # 8 * 12800
SHARD = NPAD // NC_CORES  # 12800
CHUNK = 512
NCHUNK = SHARD // CHUNK   # 25
PIECE = 5                 # chunks per input DMA
SUPER = 8                 # chunks per supergroup (4096 keys)
WIN = 8                   # keys per candidate window
L3W = 3 * 512 + 64        # 1600 window-maxes per qt per core
BROWS = B // NC_CORES     # 32 rows per core in phase 2
NCD = BROWS * K           # 1024 candidate rows in phase 2
F8SCALE = 64.0

f32 = mybir.dt.float32
bf16 = mybir.dt.bfloat16
f8 = mybir.dt.float8e4
u32 = mybir.dt.uint32
DR = mybir.MatmulPerfMode.DoubleRow

np_f8 = ml_dtypes.float8_e4m3
np_bf16 = ml_dtypes.bfloat16

_PH1 = None
_PH2 = None


def _build_phase1():
    nc = bacc.Bacc("TRN2", target_bir_lowering=False)
    keys_d = nc.dram_tensor("keys", [128, NCHUNK, 2, 2, CHUNK], f8, kind="ExternalInput")
    qT_d = nc.dram_tensor("qT", [128, 2, 2, 2, 128], f8, kind="ExternalInput")
    l3_d = nc.dram_tensor("l3", [2, 128, L3W], bf16, kind="ExternalOutput")

    with TileContext(nc) as tc:
        with (
            tc.tile_pool(name="qp", bufs=1) as qpool,
            tc.tile_pool(name="keys", bufs=3) as keyp,
            tc.tile_pool(name="lv", bufs=1) as lvp,
            tc.tile_pool(name="eb", bufs=3) as ebp,
            tc.tile_pool(name="psum", bufs=2, space="PSUM") as psump,
        ):
            qt_t = qpool.tile([128, 2, 2, 2, 128], f8, tag="q")
            nc.sync.dma_start(out=qt_t[:], in_=qT_d[:, :, :, :, :])

            pieces = {}
            for p in range(NCHUNK // PIECE):
                kt = keyp.tile([128, PIECE, 2, 2, CHUNK], f8, tag="kt")
                nc.sync.dma_start(out=kt[:], in_=keys_d[:, p * PIECE:(p + 1) * PIECE])
                pieces[p] = kt

            # L1 buffers per (qt, supergroup); L3 output per qt
            sl1 = {}
            l3o = [lvp.tile([128, L3W], bf16, tag=f"l3o{qt}", name=f"l3o{qt}")
                   for qt in range(2)]

            # full psum tiles: ft = 0..5 cover chunks 4ft..4ft+3 ([128, 2048]);
            # then chunk 24 in a quarter tile.  PSUM dual-buffered across qt.
            for ft in range(6):
                for qt in range(2):
                    ps = psump.tile([128, 2048], f32, tag="ps")
                    for ci in range(4):
                        ch = 4 * ft + ci
                        kt = pieces[ch // PIECE]
                        lc = ch % PIECE
                        for m in range(2):
                            nc.tensor.matmul(
                                ps[:, ci * 512:(ci + 1) * 512],
                                lhsT=qt_t[:, qt, m, :, :],
                                rhs=kt[:, lc, m, :, :],
                                start=(m == 0), stop=(m == 1),
                                perf_mode=DR,
                            )
                    s = ft // 2
                    half = ft % 2
                    if half == 0:
                        sl1[(qt, s)] = lvp.tile([128, 2048], bf16, tag=f"sl1_{qt}",
                                                name=f"sl1_{qt}_{s}")
                    dst = sl1[(qt, s)][:, half * 1024:(half + 1) * 1024]
                    # L1: pair cols (g, g+1024) of the tile -> [128, 1024] bf16
                    if ft == 0:
                        # DVE-direct windowed reduce (one PSUM input)
                        pv = ps[:].rearrange("p (two x) -> p x two", two=2)
                        nc.vector.reduce_max(out=dst, in_=pv,
                                             axis=mybir.AxisListType.X)
                    else:
                        # ACT evict to bf16, then DVE pairwise max at 2x
                        eb = ebp.tile([128, 2048], bf16, tag="eb")
                        nc.scalar.copy(out=eb[:], in_=ps[:])
                        nc.vector.tensor_max(out=dst, in0=eb[:, 0:1024],
                                             in1=eb[:, 1024:2048])
                    if half == 1:
                        t1 = sl1[(qt, s)]
                        l2 = lvp.tile([128, 1024], bf16, tag=f"sl2_{qt}",
                                      name=f"sl2_{qt}_{s}")
                        nc.vector.tensor_max(out=l2[:], in0=t1[:, 0:1024],
                                             in1=t1[:, 1024:2048])
                        nc.vector.tensor_max(out=l3o[qt][:, s * 512:(s + 1) * 512],
                                             in0=l2[:, 0:512], in1=l2[:, 512:1024])
            # chunk 24 (small supergroup)
            for qt in range(2):
                ps = psump.tile([128, 2048], f32, tag="ps")
                kt = pieces[24 // PIECE]
                for m in range(2):
                    nc.tensor.matmul(
                        ps[:, 0:512],
                        lhsT=qt_t[:, qt, m, :, :],
                        rhs=kt[:, 24 % PIECE, m, :, :],
                        start=(m == 0), stop=(m == 1),
                        perf_mode=DR,
                    )
                sm1 = lvp.tile([128, 256], bf16, tag=f"sm1_{qt}", name=f"sm1_{qt}")
                pv = ps[:, 0:512].rearrange("p (two x) -> p x two", two=2)
                nc.vector.reduce_max(out=sm1[:], in_=pv, axis=mybir.AxisListType.X)
                sm2 = lvp.tile([128, 128], bf16, tag=f"sm2_{qt}", name=f"sm2_{qt}")
                nc.vector.tensor_max(out=sm2[:], in0=sm1[:, 0:128],
                                     in1=sm1[:, 128:256])
                nc.vector.tensor_max(out=l3o[qt][:, 1536:1600],
                                     in0=sm2[:, 0:64], in1=sm2[:, 64:128])
            for qt in range(2):
                nc.sync.dma_start(out=l3_d[qt], in_=l3o[qt][:])
    nc.finalize()
    return nc


def _build_phase2():
    nc = bacc.Bacc("TRN2", target_bir_lowering=False)
    # one bf16 weight/constant panel shared by all cores + per-core tensors
    # panel columns:
    #   Wq   [0, 1024)      [p, dc*256 + a_col]   (a_col = at*128 + a)
    #   Wm   [1024, 2048)
    #   Ws   [2048, 2050)   [p, at]
    #   Wc   [2050, 2850)   [p, m*100 + j]
    #   mask [2850, 3106)   [p, t*32 + qc] = 1 if qc == 4t + p//32 else 0
    #   onec [3106, 3107)   all-ones column
    #   oner [3107, 3235)   row of ones on partition 0 only
    #   qT   [3235, 3363)   [p, dc*32 + q]  (relu'd query, bf16)
    PW = 3363
    OWQ, OWM, OWS, OWC, OMSK, OONE, OONR, OQT = 0, 1024, 2048, 2050, 2850, 3106, 3107, 3235
    panel_d = nc.dram_tensor("panel", [128, PW], bf16, kind="ExternalInput")
    bqm_d = nc.dram_tensor("bqm", [128, 2], f32, kind="ExternalInput")
    knnT_d = nc.dram_tensor("knnT", [128, 4, NCD], bf16, kind="ExternalInput")
    knn_d = nc.dram_tensor("knn", [128, 8, D], bf16, kind="ExternalInput")
    out_d = nc.dram_tensor("out", [BROWS, C], f32, kind="ExternalOutput")

    with TileContext(nc) as tc:
        with (
            tc.tile_pool(name="sb", bufs=1) as sb,
            tc.tile_pool(name="ps", bufs=1, space="PSUM") as psp,
        ):
            panel = sb.tile([128, PW], bf16, tag="panel")
            nc.sync.dma_start(out=panel[:], in_=panel_d[:, :])
            bqm = sb.tile([128, 2], f32, tag="bqm")
            nc.sync.dma_start(out=bqm[:], in_=bqm_d[:, :])
            knnT = sb.tile([128, 4, NCD], bf16, tag="knnT")
            nc.sync.dma_start(out=knnT[:], in_=knnT_d[:, :, :])
            knn = sb.tile([128, 8, D], bf16, tag="knn")
            nc.sync.dma_start(out=knn[:], in_=knn_d[:, :, :])

            Wq = lambda dc, at: panel[:, OWQ + dc * 256 + at * 128: OWQ + dc * 256 + (at + 1) * 128]
            Wm = lambda dc, at: panel[:, OWM + dc * 256 + at * 128: OWM + dc * 256 + (at + 1) * 128]
            Ws = lambda at: panel[:, OWS + at: OWS + at + 1]
            Wc = lambda m: panel[:, OWC + m * C: OWC + (m + 1) * C]
            mask = panel[:, OMSK:OMSK + 256]
            onec = panel[:, OONE:OONE + 1]
            oner = panel[0:1, OONR:OONR + 128]
            qT = lambda dc: panel[:, OQT + dc * 32: OQT + (dc + 1) * 32]

            # psum small1: qp0 [0:32], qp1 [32:64], sc [64:72]
            small1 = psp.tile([128, 72], f32, tag="small1")
            # psum small2: den [0:32](p0), rbc [32:64], outc [64:164]
            small2 = psp.tile([128, 164], f32, tag="small2")
            attps = psp.tile([128, 128], f32, tag="attps")

            # ---- qprojT [at][128, 32] ----
            for at in range(2):
                for dc in range(4):
                    nc.tensor.matmul(small1[:, at * 32:(at + 1) * 32],
                                     lhsT=Wq(dc, at), rhs=qT(dc),
                                     start=(dc == 0), stop=(dc == 3))
            qsb = sb.tile([128, 64], f32, tag="qsb")
            nc.scalar.copy(out=qsb[:], in_=small1[:, 0:64])

            # ---- kprojT + add qproj + tanh -> hT[at] bf16 [128, 1024] ----
            hT = []
            for at in range(2):
                kp = psp.tile([128, NCD], f32, tag="kp", bufs=2)
                for dc in range(4):
                    for half in range(2):
                        nc.tensor.matmul(kp[:, half * 512:(half + 1) * 512],
                                         lhsT=Wm(dc, at),
                                         rhs=knnT[:, dc, half * 512:(half + 1) * 512],
                                         start=(dc == 0), stop=(dc == 3))
                qb = qsb[:, at * 32:(at + 1) * 32][:, :, None].to_broadcast(
                    [128, BROWS, K])
                nc.vector.tensor_tensor(
                    kp[:].rearrange("p (q k) -> p q k", k=K),
                    kp[:].rearrange("p (q k) -> p q k", k=K),
                    qb, mybir.AluOpType.add)
                h = sb.tile([128, NCD], bf16, tag=f"hT{at}", name=f"hT{at}")
                nc.scalar.activation(h[:], kp[:], mybir.ActivationFunctionType.Tanh,
                                     bias=bqm[:, at:at + 1])
                hT.append(h)

            # ---- scores candidate-major: sc[p, t] = h[:, t*128+p] . Ws ----
            for t in range(8):
                for at in range(2):
                    nc.tensor.matmul(small1[:, 64 + t:64 + t + 1],
                                     lhsT=hT[at][:, t * 128:(t + 1) * 128],
                                     rhs=Ws(at), start=(at == 0), stop=(at == 1))
            e_col = sb.tile([128, 8], f32, tag="e_col")
            nc.scalar.activation(e_col[:], small1[:, 64:72],
                                 mybir.ActivationFunctionType.Exp)

            # ---- w2[p, t, qc] = e_col[p, t] * mask[p, t*32+qc] (bf16) ----
            w2 = sb.tile([128, 8, BROWS], bf16, tag="w2")
            eb = e_col[:, :, None].to_broadcast([128, 8, BROWS])
            nc.vector.tensor_tensor(w2[:], eb,
                                    mask.rearrange("p (t q) -> p t q", q=BROWS),
                                    mybir.AluOpType.mult)

            # ---- den row + reciprocal + broadcast ----
            for t in range(8):
                nc.tensor.matmul(small2[0:1, 0:32], lhsT=onec, rhs=w2[:, t, :],
                                 start=(t == 0), stop=(t == 7))
            rrow = sb.tile([1, 32], bf16, tag="rrow")
            with nc.allow_low_precision(reason="softmax denom recip to bf16 for matmul broadcast"):
                nc.vector.reciprocal(rrow[:], small2[0:1, 0:32])
            nc.tensor.matmul(small2[:, 32:64], lhsT=oner, rhs=rrow[:],
                             start=True, stop=True)
            rbsb = sb.tile([128, 32], f32, tag="rbsb")
            nc.scalar.copy(out=rbsb[:], in_=small2[:, 32:64])

            # ---- attT[p, dc*32+qc] = sum_t knn[:, t, dc-block] ^T w2 ----
            for dc in range(4):
                for t in range(8):
                    nc.tensor.matmul(attps[:, dc * 32:(dc + 1) * 32],
                                     lhsT=knn[:, t, dc * 128:(dc + 1) * 128],
                                     rhs=w2[:, t, :], start=(t == 0), stop=(t == 7))
            attbf = sb.tile([128, 4, 32], bf16, tag="attbf")
            rb = rbsb[:][:, None, :].to_broadcast([128, 4, 32])
            nc.vector.tensor_tensor(attbf[:],
                                    attps[:].rearrange("p (d q) -> p d q", q=32),
                                    rb, mybir.AluOpType.mult)

            # ---- classifier ----
            for m in range(8):
                lhsT = qT(m) if m < 4 else attbf[:, m - 4, :]
                nc.tensor.matmul(small2[0:BROWS, 64:164], lhsT=lhsT, rhs=Wc(m),
                                 start=(m == 0), stop=(m == 7))
            out_sb = sb.tile([BROWS, C], f32, tag="out_sb")
            nc.scalar.copy(out=out_sb[:], in_=small2[0:BROWS, 64:164])
            nc.sync.dma_start(out=out_d[:, :], in_=out_sb[:])
    nc.finalize()
    return nc


def _phase1_nc():
    global _PH1
    if _PH1 is None:
        _PH1 = _build_phase1()
    return _PH1


def _phase2_nc():
    global _PH2
    if _PH2 is None:
        _PH2 = _build_phase2()
    return _PH2


def kernel(query_feat, memory_keys, Wq, bq, Wm, bm, Ws, bs, Wc, bc):
    query_feat = np.asarray(query_feat, np.float32)
    memory_keys = np.asarray(memory_keys, np.float32)

    # ---- host prep: normalize + fp8 quantize + shard/layout keys ----
    kn = np.sqrt((memory_keys ** 2).sum(axis=1))
    khat = memory_keys * (F8SCALE / np.maximum(kn, 1e-8))[:, None]
    k8 = np.zeros((NPAD, D), np_f8)
    k8[:N] = khat.astype(np_f8)
    # keys_d[p, ch, m, t, j] = k8[c*SHARD + ch*CHUNK + j, m*256 + t*128 + p]
    kv = k8.reshape(NC_CORES, NCHUNK, CHUNK, 2, 2, 128)   # c ch j m t p
    kv = kv.transpose(0, 5, 1, 3, 4, 2)                   # c p ch m t j
    kv = np.ascontiguousarray(kv)

    q32 = np.maximum(query_feat, 0)
    qn = np.sqrt((q32 ** 2).sum(axis=1))
    qhat = q32 * (F8SCALE / np.maximum(qn, 1e-8))[:, None]
    q8 = qhat.astype(np_f8)
    # qT_d[p, qt, m, t, qq] = q8[qt*128+qq, m*256+t*128+p]
    qv = q8.reshape(2, 128, 2, 2, 128).transpose(4, 0, 2, 3, 1)
    qv = np.ascontiguousarray(qv)

    ph1 = _phase1_nc()
    in_maps = [{"keys": kv[c], "qT": qv} for c in range(NC_CORES)]
    res1 = run_bass_kernel_spmd(ph1, in_maps, core_ids=list(range(NC_CORES)))

    # ---- host merge: top-8 windows per supergroup, expand, exact re-score ----
    # vals[b, c, :] = window maxes for query b, core c
    vals = np.empty((B, NC_CORES, L3W), np.float32)
    for c in range(NC_CORES):
        l3 = res1.results[c]["l3"]                      # [2, 128, 1600] bf16
        v = np.asarray(l3).astype(np.float32)
        vals[:128, c] = v[0]
        vals[128:, c] = v[1]

    # full supers: window (c, s, i) -> keys c*SHARD + (8s+cc)*CHUNK + i
    vfull = vals[:, :, :1536].reshape(B, NC_CORES, 3, 512)
    i8 = np.argpartition(-vfull, 7, axis=3)[:, :, :, :8]          # [B, 8, 3, 8]
    base = (np.arange(NC_CORES) * SHARD)[None, :, None, None]
    sbase = (np.arange(3) * (SUPER * CHUNK))[None, None, :, None]
    kfull = (base + sbase + i8)[..., None] + (np.arange(8) * CHUNK)[None, None, None, None, :]
    # small super: window (c, i) -> keys c*SHARD + 24*CHUNK + i + 64*m
    vsm = vals[:, :, 1536:]                                       # [B, 8, 64]
    is8 = np.argpartition(-vsm, 7, axis=2)[:, :, :8]              # [B, 8, 8]
    ksm = (np.arange(NC_CORES) * SHARD)[None, :, None, None] + 24 * CHUNK \
        + is8[..., None] + (np.arange(8) * 64)[None, None, None, :]
    cand = np.concatenate([kfull.reshape(B, -1), ksm.reshape(B, -1)], axis=1)
    cand = np.sort(cand, axis=1)                                  # ties -> lowest idx

    safe = np.minimum(cand, N - 1)
    gk = memory_keys[safe]                                        # [B, 2048, 512]
    dots = np.matmul(gk, q32[:, :, None].astype(np.float32))[:, :, 0]
    cos = dots / np.maximum(qn[:, None] * kn[safe], np.float32(1e-8))
    cos[cand >= N] = -np.inf
    order = np.argsort(-cos, axis=1, kind="stable")[:, :K]
    top_idx = np.take_along_axis(safe, order, axis=1)             # [B, 32]
    knn = memory_keys[top_idx]                                    # [B, 32, 512]

    # ---- phase 2 (batch sharded, bf16) ----
    ph2 = _phase2_nc()
    PW = 3363
    panel = np.zeros((128, PW), np.float32)
    Wq_a = np.asarray(Wq, np.float32)
    Wm_a = np.asarray(Wm, np.float32)
    Ws_a = np.asarray(Ws, np.float32).reshape(A)
    Wc_a = np.asarray(Wc, np.float32)
    # Wq/Wm: [p, dc*256 + a] = W[dc*128+p, a]
    panel[:, 0:1024] = Wq_a.reshape(4, 128, 256).transpose(1, 0, 2).reshape(128, 1024)
    panel[:, 1024:2048] = Wm_a.reshape(4, 128, 256).transpose(1, 0, 2).reshape(128, 1024)
    panel[:, 2048:2050] = Ws_a.reshape(2, 128).T
    panel[:, 2050:2850] = Wc_a.reshape(8, 128, C).transpose(1, 0, 2).reshape(128, 800)
    msk = np.zeros((128, 8, 32), np.float32)
    p_arr = np.arange(128)
    for t in range(8):
        msk[p_arr, t, 4 * t + p_arr // 32] = 1.0
    panel[:, 2850:3106] = msk.reshape(128, 256)
    panel[:, 3106] = 1.0
    panel[0, 3107:3235] = 1.0
    bqm = (np.asarray(bq, np.float32) + np.asarray(bm, np.float32)).reshape(2, 128).T
    bqm = np.ascontiguousarray(bqm)

    in_maps2 = []
    for c in range(NC_CORES):
        rows = slice(c * BROWS, (c + 1) * BROWS)
        pc = panel.copy()
        # qT: [p, dc*32 + q] = relu(q)[c*32+q, dc*128+p]
        pc[:, 3235:3363] = q32[rows].T.reshape(4, 128, 32).transpose(1, 0, 2).reshape(128, 128)
        knn_c = knn[rows].reshape(NCD, D)
        # knnT[p, dc, cand] = knn_c[cand, dc*128+p]
        knnT_c = knn_c.T.reshape(4, 128, NCD).transpose(1, 0, 2)
        # knn[p, t, d] = knn_c[t*128+p, d]
        knnr_c = knn_c.reshape(8, 128, D).transpose(1, 0, 2)
        in_maps2.append({
            "panel": pc.astype(np_bf16),
            "bqm": bqm,
            "knnT": np.ascontiguousarray(knnT_c).astype(np_bf16),
            "knn": np.ascontiguousarray(knnr_c).astype(np_bf16),
        })
    res2 = run_bass_kernel_spmd(ph2, in_maps2, core_ids=list(range(NC_CORES)))
    out = np.concatenate([np.asarray(res2.results[c]["out"]) for c in range(NC_CORES)],
                         axis=0)
    return (out + np.asarray(bc, np.float32)[None, :]).astype(np.float32)


# revision 9
# speedup vs baseline: 2.4943x; 1.1632x over previous
"""Trainium2 Bass kernel for retrieval-knn attention classifier (nn_MA_51866025067137).

Strategy (8 NeuronCores):
  Phase 1 — memory_keys sharded along N (12800 keys/core, padded 100000->102400
  with zero rows).  Keys and queries are L2-normalized on host and cast to
  fp8-e4m3 (x64 scale).  Each core ranks all 256 queries against its shard with
  DoubleRow fp8 matmuls (2 k-tiles per matmul, 256-deep contraction).  The
  Activation engine evicts sim tiles from PSUM to bf16 SBUF (the DVE handles a
  couple of tiles directly via windowed reduce_max); the DVE then folds three
  levels of pairwise tensor_max (bf16 at 2x) yielding one bf16 max per 8-key
  window.  The [128, 1600] window-max arrays stream out per supergroup; the
  host picks the top-8 windows per 4096-key supergroup, expands each window to
  its 8 keys, re-scores candidates exactly in fp32, and takes the global
  top-32.
  Phase 2 — batch sharded (32 queries/core): memory-attention module
  (tanh(qWq + knnWm + b) -> scores -> softmax -> weighted sum) and classifier
  in bf16.  Scores are computed candidate-major so softmax needs no transpose
  bounce; weights stream in a fused panel; knnT streams per-dc-chunk from the
  Pool queue so the kproj matmuls start early; dummy matmuls warm the PE
  p-state during the DMA lead-in.
"""

import numpy as np
import ml_dtypes

import concourse.bacc as bacc
import concourse.mybir as mybir
from concourse.tile import TileContext
from concourse.bass_utils import run_bass_kernel_spmd

# problem dims (hardcoded per harness contract)
B, N, D = 256, 100000, 512
A, C, K = 256, 100, 32
NC_CORES = 8
NPAD = 102400             # 8 * 12800
SHARD = NPAD // NC_CORES  # 12800
CHUNK = 512
NCHUNK = SHARD // CHUNK   # 25
SUPER = 8                 # chunks per supergroup (4096 keys)
WIN = 8                   # keys per candidate window
L3W = 3 * 512 + 64        # 1600 window-maxes per qt per core
BROWS = B // NC_CORES     # 32 rows per core in phase 2
NCD = BROWS * K           # 1024 candidate rows in phase 2
F8SCALE = 64.0

# phase-1 schedule knobs (tuned against the instruction cost model)
CHUNK_ORDER = [24] + list(range(24))      # dram slot order; chunk 24 first
PIECE_PLAN = [2] * 12 + [1]               # chunks per key DMA
KEY_BUFS = 6
EB_BUFS = 4
DIRECT = {(5, 0)}                         # (ft, qt) tiles reduced by DVE
P2_WARMUP = 9                             # phase-2 PE warmup matmuls

f32 = mybir.dt.float32
bf16 = mybir.dt.bfloat16
f8 = mybir.dt.float8e4
DR = mybir.MatmulPerfMode.DoubleRow

np_f8 = ml_dtypes.float8_e4m3
np_bf16 = ml_dtypes.bfloat16

_PH1 = None
_PH2 = None


def _build_phase1():
    nc = bacc.Bacc("TRN2", target_bir_lowering=False)
    keys_d = nc.dram_tensor("keys", [128, NCHUNK, 2, 2, CHUNK], f8, kind="ExternalInput")
    qT_d = nc.dram_tensor("qT", [128, 2, 2, 2, 128], f8, kind="ExternalInput")
    l3_d = nc.dram_tensor("l3", [2, 128, L3W], bf16, kind="ExternalOutput")

    with TileContext(nc) as tc:
        with (
            tc.tile_pool(name="qp", bufs=1) as qpool,
            tc.tile_pool(name="keys", bufs=KEY_BUFS) as keyp,
            tc.tile_pool(name="lv", bufs=1) as lvp,
            tc.tile_pool(name="eb", bufs=EB_BUFS) as ebp,
            tc.tile_pool(name="psum", bufs=2, space="PSUM") as psump,
        ):
            qt_t = qpool.tile([128, 2, 2, 2, 128], f8, tag="q")
            nc.gpsimd.dma_start(out=qt_t[:], in_=qT_d[:, :, :, :, :])
            loaded = {}
            lo = 0
            for cnt in PIECE_PLAN:
                hi = lo + cnt
                kt = keyp.tile([128, cnt, 2, 2, CHUNK], f8, tag="kt", name="kt")
                nc.gpsimd.dma_start(out=kt[:], in_=keys_d[:, lo:hi])
                for s in range(lo, hi):
                    loaded[CHUNK_ORDER[s]] = (kt, s - lo)
                lo = hi

            sl1 = {}
            l3o = [lvp.tile([128, L3W], bf16, tag=f"l3o{qt}", name=f"l3o{qt}")
                   for qt in range(2)]

            def fold(out, in0, in1):
                nc.vector.tensor_max(out=out, in0=in0, in1=in1)

            def mm(ps, qt, chunks):
                for ci, ch in enumerate(chunks):
                    kt, lc = loaded[ch]
                    for m in range(2):
                        nc.tensor.matmul(
                            ps[:, ci * 512:(ci + 1) * 512],
                            lhsT=qt_t[:, qt, m, :, :],
                            rhs=kt[:, lc, m, :, :],
                            start=(m == 0), stop=(m == 1),
                            perf_mode=DR,
                        )

            # small supergroup (chunk 24) first: its data is in piece 0
            for qt in range(2):
                ps = psump.tile([128, 2048], f32, tag="ps", name="ps_sm")
                mm(ps, qt, [24])
                sm1 = lvp.tile([128, 256], bf16, tag=f"sm1_{qt}", name=f"sm1_{qt}")
                pv = ps[:, 0:512].rearrange("p (two x) -> p x two", two=2)
                nc.vector.reduce_max(out=sm1[:], in_=pv, axis=mybir.AxisListType.X)
                sm2 = lvp.tile([128, 128], bf16, tag=f"sm2_{qt}", name=f"sm2_{qt}")
                fold(sm2[:], sm1[:, 0:128], sm1[:, 128:256])
                fold(l3o[qt][:, 1536:1600], sm2[:, 0:64], sm2[:, 64:128])
                nc.sync.dma_start(out=l3_d[qt, :, 1536:1600],
                                  in_=l3o[qt][:, 1536:1600])

            # full psum tiles: ft covers chunks 4ft..4ft+3 ([128, 2048], 4 banks)
            for ft in range(6):
                for qt in range(2):
                    ps = psump.tile([128, 2048], f32, tag="ps", name="ps_ft")
                    mm(ps, qt, [4 * ft, 4 * ft + 1, 4 * ft + 2, 4 * ft + 3])
                    s = ft // 2
                    half = ft % 2
                    if half == 0:
                        sl1[(qt, s)] = lvp.tile([128, 2048], bf16, tag=f"sl1_{qt}",
                                                name=f"sl1_{qt}_{s}")
                    dst = sl1[(qt, s)][:, half * 1024:(half + 1) * 1024]
                    if (ft, qt) in DIRECT:
                        pv = ps[:].rearrange("p (two x) -> p x two", two=2)
                        nc.vector.reduce_max(out=dst, in_=pv,
                                             axis=mybir.AxisListType.X)
                    else:
                        eb = ebp.tile([128, 2048], bf16, tag="eb", name="eb")
                        nc.scalar.copy(out=eb[:], in_=ps[:])
                        fold(dst, eb[:, 0:1024], eb[:, 1024:2048])
                    if half == 1:
                        t1 = sl1[(qt, s)]
                        l2 = lvp.tile([128, 1024], bf16, tag=f"sl2_{qt}",
                                      name=f"sl2_{qt}_{s}")
                        fold(l2[:], t1[:, 0:1024], t1[:, 1024:2048])
                        fold(l3o[qt][:, s * 512:(s + 1) * 512],
                             l2[:, 0:512], l2[:, 512:1024])
                        nc.sync.dma_start(out=l3_d[qt, :, s * 512:(s + 1) * 512],
                                          in_=l3o[qt][:, s * 512:(s + 1) * 512])
    nc.finalize()
    return nc


def _build_phase2():
    nc = bacc.Bacc("TRN2", target_bir_lowering=False)
    # bf16 weight/constant panel shared by all cores + per-core tensors
    # panel columns:
    #   Wq   [0, 1024)      [p, dc*256 + at*128 + a]
    #   Wm   [1024, 2048)
    #   Ws   [2048, 2050)   [p, at]
    #   Wc   [2050, 2850)   [p, m*100 + j]
    #   mask [2850, 3106)   [p, t*32 + qc] = 1 if qc == 4t + p//32 else 0
    #   onec [3106, 3107)   all-ones column
    #   oner [3107, 3235)   row of ones on partition 0 only
    #   qT   [3235, 3363)   [p, dc*32 + q]  (relu'd query, bf16)
    PW = 3363
    OWQ, OWM, OWS, OWC = 0, 1024, 2048, 2050
    OMSK, OONE, OONR, OQT = 2850, 3106, 3107, 3235
    panel_d = nc.dram_tensor("panel", [128, PW], bf16, kind="ExternalInput")
    bqm_d = nc.dram_tensor("bqm", [128, 2], f32, kind="ExternalInput")
    knnT_d = nc.dram_tensor("knnT", [128, 4, NCD], bf16, kind="ExternalInput")
    knn_d = nc.dram_tensor("knn", [128, 8, D], bf16, kind="ExternalInput")
    out_d = nc.dram_tensor("out", [BROWS, C], f32, kind="ExternalOutput")

    with TileContext(nc) as tc:
        with (
            tc.tile_pool(name="sb", bufs=1) as sb,
            tc.tile_pool(name="ps", bufs=1, space="PSUM") as psp,
        ):
            panel = sb.tile([128, PW], bf16, tag="panel")
            bqm = sb.tile([128, 2], f32, tag="bqm")
            knnT = sb.tile([128, 4, NCD], bf16, tag="knnT")
            knn = sb.tile([128, 8, D], bf16, tag="knn")
            # Wm first (kproj), knnT per-dc on the Pool queue, rest after
            nc.sync.dma_start(out=panel[:, OWM:OWM + 1024],
                              in_=panel_d[:, OWM:OWM + 1024])
            for dc in range(4):
                nc.gpsimd.dma_start(out=knnT[:, dc, :], in_=knnT_d[:, dc, :])
            nc.sync.dma_start(out=panel[:, 0:OWM], in_=panel_d[:, 0:OWM])
            nc.sync.dma_start(out=panel[:, OWM + 1024:], in_=panel_d[:, OWM + 1024:])
            nc.sync.dma_start(out=bqm[:], in_=bqm_d[:, :])
            nc.gpsimd.dma_start(out=knn[:], in_=knn_d[:, :, :])

            Wq = lambda dc, at: panel[:, OWQ + dc * 256 + at * 128: OWQ + dc * 256 + (at + 1) * 128]
            Wm = lambda dc, at: panel[:, OWM + dc * 256 + at * 128: OWM + dc * 256 + (at + 1) * 128]
            Ws = lambda at: panel[:, OWS + at: OWS + at + 1]
            Wc = lambda m: panel[:, OWC + m * C: OWC + (m + 1) * C]
            mask = panel[:, OMSK:OMSK + 256]
            onec = panel[:, OONE:OONE + 1]
            oner = panel[0:1, OONR:OONR + 128]
            qT = lambda dc: panel[:, OQT + dc * 32: OQT + (dc + 1) * 32]

            # psum small1: qp0 [0:32], qp1 [32:64], sc [64:72]
            small1 = psp.tile([128, 72], f32, tag="small1")
            # psum small2: den [0:32](p0), rbc [32:64], outc [64:164]
            small2 = psp.tile([128, 164], f32, tag="small2")
            attps = psp.tile([128, 128], f32, tag="attps")

            # PE p-state warmup on scratch data during the DMA lead-in
            wsb = sb.tile([128, 512], bf16, tag="wsb")
            nc.vector.memset(wsb[:].bitcast(mybir.dt.uint16), 0)
            wps = psp.tile([128, 512], f32, tag="wps")
            for _ in range(P2_WARMUP):
                nc.tensor.matmul(wps[:], lhsT=wsb[:, 0:128], rhs=wsb[:],
                                 start=True, stop=True, skip_group_check=True)

            # ---- qprojT [at][128, 32] ----
            for at in range(2):
                for dc in range(4):
                    nc.tensor.matmul(small1[:, at * 32:(at + 1) * 32],
                                     lhsT=Wq(dc, at), rhs=qT(dc),
                                     start=(dc == 0), stop=(dc == 3))
            qsb = sb.tile([128, 64], f32, tag="qsb")
            nc.scalar.copy(out=qsb[:], in_=small1[:, 0:64])

            # ---- kprojT (dc-major to chase the knnT stream) ----
            kps = []
            for at in range(2):
                kps.append(psp.tile([128, NCD], f32, tag="kp", bufs=2,
                                    name=f"kp{at}"))
            for dc in range(4):
                for at in range(2):
                    for half in range(2):
                        nc.tensor.matmul(kps[at][:, half * 512:(half + 1) * 512],
                                         lhsT=Wm(dc, at),
                                         rhs=knnT[:, dc, half * 512:(half + 1) * 512],
                                         start=(dc == 0), stop=(dc == 3))
            # ---- + qproj broadcast, tanh -> hT[at] bf16 [128, 1024] ----
            hT = []
            for at in range(2):
                kp = kps[at]
                qb = qsb[:, at * 32:(at + 1) * 32][:, :, None].to_broadcast(
                    [128, BROWS, K])
                nc.vector.tensor_tensor(
                    kp[:].rearrange("p (q k) -> p q k", k=K),
                    kp[:].rearrange("p (q k) -> p q k", k=K),
                    qb, mybir.AluOpType.add)
                h = sb.tile([128, NCD], bf16, tag=f"hT{at}", name=f"hT{at}")
                nc.scalar.activation(h[:], kp[:], mybir.ActivationFunctionType.Tanh,
                                     bias=bqm[:, at:at + 1])
                hT.append(h)

            # ---- scores candidate-major: sc[p, t] = h[:, t*128+p] . Ws ----
            for t in range(8):
                for at in range(2):
                    nc.tensor.matmul(small1[:, 64 + t:64 + t + 1],
                                     lhsT=hT[at][:, t * 128:(t + 1) * 128],
                                     rhs=Ws(at), start=(at == 0), stop=(at == 1))
            e_col = sb.tile([128, 8], f32, tag="e_col")
            nc.scalar.activation(e_col[:], small1[:, 64:72],
                                 mybir.ActivationFunctionType.Exp)

            # ---- w2[p, t, qc] = e_col[p, t] * mask[p, t*32+qc] (bf16) ----
            w2 = sb.tile([128, 8, BROWS], bf16, tag="w2")
            eb = e_col[:, :, None].to_broadcast([128, 8, BROWS])
            nc.vector.tensor_tensor(w2[:], eb,
                                    mask.rearrange("p (t q) -> p t q", q=BROWS),
                                    mybir.AluOpType.mult)

            # ---- den row + reciprocal + broadcast ----
            for t in range(8):
                nc.tensor.matmul(small2[0:1, 0:32], lhsT=onec, rhs=w2[:, t, :],
                                 start=(t == 0), stop=(t == 7))
            rrow = sb.tile([1, 32], bf16, tag="rrow")
            with nc.allow_low_precision(reason="softmax denom recip to bf16 for matmul broadcast"):
                nc.vector.reciprocal(rrow[:], small2[0:1, 0:32])
            nc.tensor.matmul(small2[:, 32:64], lhsT=oner, rhs=rrow[:],
                             start=True, stop=True)
            rbsb = sb.tile([128, 32], f32, tag="rbsb")
            nc.scalar.copy(out=rbsb[:], in_=small2[:, 32:64])

            # ---- attT[p, dc*32+qc] = sum_t knn[:, t, dc-block] ^T w2 ----
            for dc in range(4):
                for t in range(8):
                    nc.tensor.matmul(attps[:, dc * 32:(dc + 1) * 32],
                                     lhsT=knn[:, t, dc * 128:(dc + 1) * 128],
                                     rhs=w2[:, t, :], start=(t == 0), stop=(t == 7))
            attbf = sb.tile([128, 4, 32], bf16, tag="attbf")
            rb = rbsb[:][:, None, :].to_broadcast([128, 4, 32])
            nc.vector.tensor_tensor(attbf[:],
                                    attps[:].rearrange("p (d q) -> p d q", q=32),
                                    rb, mybir.AluOpType.mult)

            # ---- classifier ----
            for m in range(8):
                lhsT = qT(m) if m < 4 else attbf[:, m - 4, :]
                nc.tensor.matmul(small2[0:BROWS, 64:164], lhsT=lhsT, rhs=Wc(m),
                                 start=(m == 0), stop=(m == 7))
            out_sb = sb.tile([BROWS, C], f32, tag="out_sb")
            nc.scalar.copy(out=out_sb[:], in_=small2[0:BROWS, 64:164])
            nc.sync.dma_start(out=out_d[:, :], in_=out_sb[:])
    nc.finalize()
    return nc


def _phase1_nc():
    global _PH1
    if _PH1 is None:
        _PH1 = _build_phase1()
    return _PH1


def _phase2_nc():
    global _PH2
    if _PH2 is None:
        _PH2 = _build_phase2()
    return _PH2


def kernel(query_feat, memory_keys, Wq, bq, Wm, bm, Ws, bs, Wc, bc):
    query_feat = np.asarray(query_feat, np.float32)
    memory_keys = np.asarray(memory_keys, np.float32)

    # ---- host prep: normalize + fp8 quantize + shard/layout keys ----
    kn = np.sqrt((memory_keys ** 2).sum(axis=1))
    khat = memory_keys * (F8SCALE / np.maximum(kn, 1e-8))[:, None]
    k8 = np.zeros((NPAD, D), np_f8)
    k8[:N] = khat.astype(np_f8)
    # keys_d[p, slot, m, t, j] = k8[c*SHARD + CHUNK_ORDER[slot]*CHUNK + j,
    #                               m*256 + t*128 + p]
    kv = k8.reshape(NC_CORES, NCHUNK, CHUNK, 2, 2, 128)   # c ch j m t p
    kv = kv[:, CHUNK_ORDER]                               # slot reorder
    kv = kv.transpose(0, 5, 1, 3, 4, 2)                   # c p slot m t j
    kv = np.ascontiguousarray(kv)

    q32 = np.maximum(query_feat, 0)
    qn = np.sqrt((q32 ** 2).sum(axis=1))
    qhat = q32 * (F8SCALE / np.maximum(qn, 1e-8))[:, None]
    q8 = qhat.astype(np_f8)
    # qT_d[p, qt, m, t, qq] = q8[qt*128+qq, m*256+t*128+p]
    qv = q8.reshape(2, 128, 2, 2, 128).transpose(4, 0, 2, 3, 1)
    qv = np.ascontiguousarray(qv)

    ph1 = _phase1_nc()
    in_maps = [{"keys": kv[c], "qT": qv} for c in range(NC_CORES)]
    res1 = run_bass_kernel_spmd(ph1, in_maps, core_ids=list(range(NC_CORES)))

    # ---- host merge: top-8 windows per supergroup, expand, exact re-score ----
    vals = np.empty((B, NC_CORES, L3W), np.float32)
    for c in range(NC_CORES):
        v = np.asarray(res1.results[c]["l3"]).astype(np.float32)  # [2, 128, 1600]
        vals[:128, c] = v[0]
        vals[128:, c] = v[1]

    # full supers: window (c, s, i) -> keys c*SHARD + (8s+cc)*CHUNK + i
    vfull = vals[:, :, :1536].reshape(B, NC_CORES, 3, 512)
    i8 = np.argpartition(-vfull, 7, axis=3)[:, :, :, :8]          # [B, 8, 3, 8]
    base = (np.arange(NC_CORES) * SHARD)[None, :, None, None]
    sbase = (np.arange(3) * (SUPER * CHUNK))[None, None, :, None]
    kfull = (base + sbase + i8)[..., None] + (np.arange(8) * CHUNK)[None, None, None, None, :]
    # small super: window (c, i) -> keys c*SHARD + 24*CHUNK + i + 64*m
    vsm = vals[:, :, 1536:]                                       # [B, 8, 64]
    is8 = np.argpartition(-vsm, 7, axis=2)[:, :, :8]              # [B, 8, 8]
    ksm = (np.arange(NC_CORES) * SHARD)[None, :, None, None] + 24 * CHUNK \
        + is8[..., None] + (np.arange(8) * 64)[None, None, None, :]
    cand = np.concatenate([kfull.reshape(B, -1), ksm.reshape(B, -1)], axis=1)
    cand = np.sort(cand, axis=1)                                  # ties -> lowest idx

    safe = np.minimum(cand, N - 1)
    gk = memory_keys[safe]                                        # [B, 2048, 512]
    dots = np.matmul(gk, q32[:, :, None].astype(np.float32))[:, :, 0]
    cos = dots / np.maximum(qn[:, None] * kn[safe], np.float32(1e-8))
    cos[cand >= N] = -np.inf
    order = np.argsort(-cos, axis=1, kind="stable")[:, :K]
    top_idx = np.take_along_axis(safe, order, axis=1)             # [B, 32]
    knn = memory_keys[top_idx]                                    # [B, 32, 512]

    # ---- phase 2 (batch sharded, bf16) ----
    ph2 = _phase2_nc()
    PW = 3363
    panel = np.zeros((128, PW), np.float32)
    Wq_a = np.asarray(Wq, np.float32)
    Wm_a = np.asarray(Wm, np.float32)
    Ws_a = np.asarray(Ws, np.float32).reshape(A)
    Wc_a = np.asarray(Wc, np.float32)
    panel[:, 0:1024] = Wq_a.reshape(4, 128, 256).transpose(1, 0, 2).reshape(128, 1024)
    panel[:, 1024:2048] = Wm_a.reshape(4, 128, 256).transpose(1, 0, 2).reshape(128, 1024)
    panel[:, 2048:2050] = Ws_a.reshape(2, 128).T
    panel[:, 2050:2850] = Wc_a.reshape(8, 128, C).transpose(1, 0, 2).reshape(128, 800)
    msk = np.zeros((128, 8, 32), np.float32)
    p_arr = np.arange(128)
    for t in range(8):
        msk[p_arr, t, 4 * t + p_arr // 32] = 1.0
    panel[:, 2850:3106] = msk.reshape(128, 256)
    panel[:, 3106] = 1.0
    panel[0, 3107:3235] = 1.0
    bqm = (np.asarray(bq, np.float32) + np.asarray(bm, np.float32)).reshape(2, 128).T
    bqm = np.ascontiguousarray(bqm)

    in_maps2 = []
    for c in range(NC_CORES):
        rows = slice(c * BROWS, (c + 1) * BROWS)
        pc = panel.copy()
        pc[:, 3235:3363] = q32[rows].T.reshape(4, 128, 32).transpose(1, 0, 2).reshape(128, 128)
        knn_c = knn[rows].reshape(NCD, D)
        knnT_c = knn_c.T.reshape(4, 128, NCD).transpose(1, 0, 2)
        knnr_c = knn_c.reshape(8, 128, D).transpose(1, 0, 2)
        in_maps2.append({
            "panel": pc.astype(np_bf16),
            "bqm": bqm,
            "knnT": np.ascontiguousarray(knnT_c).astype(np_bf16),
            "knn": np.ascontiguousarray(knnr_c).astype(np_bf16),
        })
    res2 = run_bass_kernel_spmd(ph2, in_maps2, core_ids=list(range(NC_CORES)))
    out = np.concatenate([np.asarray(res2.results[c]["out"]) for c in range(NC_CORES)],
                         axis=0)
    return (out + np.asarray(bc, np.float32)[None, :]).astype(np.float32)


# revision 11
# speedup vs baseline: 2.5809x; 1.0347x over previous
"""Trainium2 Bass kernel for retrieval-knn attention classifier (nn_MA_51866025067137).

Strategy (8 NeuronCores):
  Phase 1 — memory_keys sharded along N (12800 keys/core, padded 100000->102400
  with zero rows).  Keys and queries are L2-normalized on host and cast to
  fp8-e4m3 (x64 scale).  Each core ranks all 256 queries against its shard with
  DoubleRow fp8 matmuls (2 k-tiles per matmul, 256-deep contraction).  The
  Activation engine evicts sim tiles from PSUM to bf16 SBUF (the DVE handles a
  couple of tiles directly via windowed reduce_max); the DVE then folds three
  levels of pairwise tensor_max (bf16 at 2x) yielding one bf16 max per 8-key
  window.  The [128, 1600] window-max arrays stream out per supergroup; the
  host picks the top-8 windows per 4096-key supergroup, expands each window to
  its 8 keys, re-scores candidates exactly in fp32, and takes the global
  top-32.
  Phase 2 — batch sharded (32 queries/core): memory-attention module
  (tanh(qWq + knnWm + b) -> scores -> softmax -> weighted sum) and classifier
  in bf16.  Scores are computed candidate-major so softmax needs no transpose
  bounce; weights stream in a fused panel; knnT streams per-dc-chunk from the
  Pool queue so the kproj matmuls start early; dummy matmuls warm the PE
  p-state during the DMA lead-in.
"""

import numpy as np
import ml_dtypes

import concourse.bacc as bacc
import concourse.mybir as mybir
from concourse.tile import TileContext
from concourse.bass_utils import run_bass_kernel_spmd

# problem dims (hardcoded per harness contract)
B, N, D = 256, 100000, 512
A, C, K = 256, 100, 32
NC_CORES = 8
NPAD = 102400             # 8 * 12800
SHARD = NPAD // NC_CORES  # 12800
CHUNK = 512
NCHUNK = SHARD // CHUNK   # 25
SUPER = 8                 # chunks per supergroup (4096 keys)
WIN = 8                   # keys per candidate window
L3W = 3 * 512 + 64        # 1600 window-maxes per qt per core
BROWS = B // NC_CORES     # 32 rows per core in phase 2
NCD = BROWS * K           # 1024 candidate rows in phase 2
F8SCALE = 64.0

# phase-1 schedule knobs (tuned against the instruction cost model)
CHUNK_ORDER = [24] + list(range(24))      # dram slot order; chunk 24 first
PIECE_PLAN = [3] + [2] * 11               # chunks per key DMA
KEY_BUFS = 6
EB_BUFS = 6
DIRECT = {(1, 0), (3, 0), (5, 0)}         # (ft, qt) tiles reduced by DVE
P2_WARMUP = 2                             # phase-2 PE warmup matmuls

f32 = mybir.dt.float32
bf16 = mybir.dt.bfloat16
f8 = mybir.dt.float8e4
DR = mybir.MatmulPerfMode.DoubleRow

np_f8 = ml_dtypes.float8_e4m3
np_bf16 = ml_dtypes.bfloat16

_PH1 = None
_PH2 = None


def _build_phase1():
    nc = bacc.Bacc("TRN2", target_bir_lowering=False)
    keys_d = nc.dram_tensor("keys", [128, NCHUNK, 2, 2, CHUNK], f8, kind="ExternalInput")
    qT_d = nc.dram_tensor("qT", [128, 2, 2, 2, 128], f8, kind="ExternalInput")
    l3_d = nc.dram_tensor("l3", [2, 128, L3W], bf16, kind="ExternalOutput")

    with TileContext(nc) as tc:
        with (
            tc.tile_pool(name="qp", bufs=1) as qpool,
            tc.tile_pool(name="keys", bufs=KEY_BUFS) as keyp,
            tc.tile_pool(name="lv", bufs=1) as lvp,
            tc.tile_pool(name="eb", bufs=EB_BUFS) as ebp,
            tc.tile_pool(name="psum", bufs=2, space="PSUM") as psump,
        ):
            qt_t = qpool.tile([128, 2, 2, 2, 128], f8, tag="q")
            nc.gpsimd.dma_start(out=qt_t[:], in_=qT_d[:, :, :, :, :])
            loaded = {}
            lo = 0
            for cnt in PIECE_PLAN:
                hi = lo + cnt
                kt = keyp.tile([128, cnt, 2, 2, CHUNK], f8, tag="kt", name="kt")
                nc.gpsimd.dma_start(out=kt[:], in_=keys_d[:, lo:hi])
                for s in range(lo, hi):
                    loaded[CHUNK_ORDER[s]] = (kt, s - lo)
                lo = hi

            sl1 = {}
            l3o = [lvp.tile([128, L3W], bf16, tag=f"l3o{qt}", name=f"l3o{qt}")
                   for qt in range(2)]

            def fold(out, in0, in1):
                nc.vector.tensor_max(out=out, in0=in0, in1=in1)

            def mm(ps, qt, chunks):
                for ci, ch in enumerate(chunks):
                    kt, lc = loaded[ch]
                    for m in range(2):
                        nc.tensor.matmul(
                            ps[:, ci * 512:(ci + 1) * 512],
                            lhsT=qt_t[:, qt, m, :, :],
                            rhs=kt[:, lc, m, :, :],
                            start=(m == 0), stop=(m == 1),
                            perf_mode=DR,
                        )

            # small supergroup (chunk 24) first: its data is in piece 0
            for qt in range(2):
                ps = psump.tile([128, 2048], f32, tag="ps", name="ps_sm")
                mm(ps, qt, [24])
                sm1 = lvp.tile([128, 256], bf16, tag=f"sm1_{qt}", name=f"sm1_{qt}")
                pv = ps[:, 0:512].rearrange("p (two x) -> p x two", two=2)
                nc.vector.reduce_max(out=sm1[:], in_=pv, axis=mybir.AxisListType.X)
                sm2 = lvp.tile([128, 128], bf16, tag=f"sm2_{qt}", name=f"sm2_{qt}")
                fold(sm2[:], sm1[:, 0:128], sm1[:, 128:256])
                fold(l3o[qt][:, 1536:1600], sm2[:, 0:64], sm2[:, 64:128])
                nc.sync.dma_start(out=l3_d[qt, :, 1536:1600],
                                  in_=l3o[qt][:, 1536:1600])

            # full psum tiles: ft covers chunks 4ft..4ft+3 ([128, 2048], 4 banks)
            for ft in range(6):
                for qt in range(2):
                    ps = psump.tile([128, 2048], f32, tag="ps", name="ps_ft")
                    mm(ps, qt, [4 * ft, 4 * ft + 1, 4 * ft + 2, 4 * ft + 3])
                    s = ft // 2
                    half = ft % 2
                    if half == 0:
                        sl1[(qt, s)] = lvp.tile([128, 2048], bf16, tag=f"sl1_{qt}",
                                                name=f"sl1_{qt}_{s}")
                    dst = sl1[(qt, s)][:, half * 1024:(half + 1) * 1024]
                    if (ft, qt) in DIRECT:
                        pv = ps[:].rearrange("p (two x) -> p x two", two=2)
                        nc.vector.reduce_max(out=dst, in_=pv,
                                             axis=mybir.AxisListType.X)
                    else:
                        eb = ebp.tile([128, 2048], bf16, tag="eb", name="eb")
                        nc.scalar.copy(out=eb[:], in_=ps[:])
                        fold(dst, eb[:, 0:1024], eb[:, 1024:2048])
                    if half == 1:
                        t1 = sl1[(qt, s)]
                        l2 = lvp.tile([128, 1024], bf16, tag=f"sl2_{qt}",
                                      name=f"sl2_{qt}_{s}")
                        fold(l2[:], t1[:, 0:1024], t1[:, 1024:2048])
                        fold(l3o[qt][:, s * 512:(s + 1) * 512],
                             l2[:, 0:512], l2[:, 512:1024])
                        nc.sync.dma_start(out=l3_d[qt, :, s * 512:(s + 1) * 512],
                                          in_=l3o[qt][:, s * 512:(s + 1) * 512])
    nc.finalize()
    return nc


def _build_phase2():
    nc = bacc.Bacc("TRN2", target_bir_lowering=False)
    # bf16 weight/constant panel shared by all cores + per-core tensors
    # panel columns (PW = 3365):
    #   Wm   [0, 1024)       [p, dc*256 + at*128 + a]
    #   Ws   [1024, 1026)    [p, at]
    #   mask [1026, 1282)    [p, t*32 + qc] = 1 if qc == 4t + p//32 else 0
    #   onec [1282]          all-ones column
    #   oner [1283, 1411)    row of ones on partition 0 only
    #   bqm  [1411, 1413)    [p, at] = (bq+bm) in bf16
    #   Wq   [1413, 2437)
    #   qT   [2437, 2565)    [p, dc*32 + q]  (relu'd query, bf16)
    #   Wc   [2565, 3365)    [p, m*100 + j]
    PW = 3365
    OWM, OWS, OMSK, OONE, OONR, OBQ, OWQ, OQT, OWC = (
        0, 1024, 1026, 1282, 1283, 1411, 1413, 2437, 2565)
    SPLIT1, SPLIT2 = 1413, 2565
    panel_d = nc.dram_tensor("panel", [128, PW], bf16, kind="ExternalInput")
    knnT_d = nc.dram_tensor("knnT", [128, 4, NCD], bf16, kind="ExternalInput")
    knn_d = nc.dram_tensor("knn", [128, 8, D], bf16, kind="ExternalInput")
    out_d = nc.dram_tensor("out", [BROWS, C], f32, kind="ExternalOutput")

    with TileContext(nc) as tc:
        with (
            tc.tile_pool(name="sb", bufs=1) as sb,
            tc.tile_pool(name="ps", bufs=1, space="PSUM") as psp,
        ):
            panel = sb.tile([128, PW], bf16, tag="panel")
            knnT = sb.tile([128, 4, NCD], bf16, tag="knnT")
            knn = sb.tile([128, 8, D], bf16, tag="knn")
            # Wm + small constants first; knnT per-dc on the Pool queue;
            # Wq/qT after the knnT stream; Wc/knn late on the Pool queue
            nc.sync.dma_start(out=panel[:, 0:SPLIT1], in_=panel_d[:, 0:SPLIT1])
            for dc in range(4):
                nc.gpsimd.dma_start(out=knnT[:, dc, :], in_=knnT_d[:, dc, :])
            nc.sync.dma_start(out=panel[:, SPLIT1:SPLIT2],
                              in_=panel_d[:, SPLIT1:SPLIT2])
            nc.gpsimd.dma_start(out=knn[:], in_=knn_d[:, :, :])
            nc.gpsimd.dma_start(out=panel[:, SPLIT2:], in_=panel_d[:, SPLIT2:])

            Wq = lambda dc, at: panel[:, OWQ + dc * 256 + at * 128: OWQ + dc * 256 + (at + 1) * 128]
            Wm = lambda dc, at: panel[:, OWM + dc * 256 + at * 128: OWM + dc * 256 + (at + 1) * 128]
            Ws = lambda at: panel[:, OWS + at: OWS + at + 1]
            Wc = lambda m: panel[:, OWC + m * C: OWC + (m + 1) * C]
            mask = panel[:, OMSK:OMSK + 256]
            onec = panel[:, OONE:OONE + 1]
            oner = panel[0:1, OONR:OONR + 128]
            bqm = panel[:, OBQ:OBQ + 2]
            qT = lambda dc: panel[:, OQT + dc * 32: OQT + (dc + 1) * 32]

            # psum small1: qp0 [0:32], qp1 [32:64], sc [64:72]
            small1 = psp.tile([128, 72], f32, tag="small1")
            # psum small2: den [0:32](p0), rbc [32:64], outc [64:164]
            small2 = psp.tile([128, 164], f32, tag="small2")
            attps = psp.tile([128, 128], f32, tag="attps")

            # PE p-state warmup on scratch data during the DMA lead-in
            wsb = sb.tile([128, 512], bf16, tag="wsb")
            nc.vector.memset(wsb[:].bitcast(mybir.dt.uint16), 0)
            wps = psp.tile([128, 512], f32, tag="wps")
            for _ in range(P2_WARMUP):
                nc.tensor.matmul(wps[:], lhsT=wsb[:, 0:128], rhs=wsb[:],
                                 start=True, stop=True, skip_group_check=True)

            # ---- kprojT chasing the knnT stream; qproj interleaved ----
            kps = [psp.tile([128, NCD], f32, tag="kp", bufs=2, name=f"kp{at}")
                   for at in range(2)]
            qsb = sb.tile([128, 64], f32, tag="qsb")
            for dc in range(4):
                for at in range(2):
                    for half in range(2):
                        nc.tensor.matmul(kps[at][:, half * 512:(half + 1) * 512],
                                         lhsT=Wm(dc, at),
                                         rhs=knnT[:, dc, half * 512:(half + 1) * 512],
                                         start=(dc == 0), stop=(dc == 3))
                if dc == 1:
                    for at in range(2):
                        for qdc in range(4):
                            nc.tensor.matmul(small1[:, at * 32:(at + 1) * 32],
                                             lhsT=Wq(qdc, at), rhs=qT(qdc),
                                             start=(qdc == 0), stop=(qdc == 3))
                    nc.scalar.copy(out=qsb[:], in_=small1[:, 0:64])

            # ---- + qproj broadcast, tanh -> hT[at] bf16 [128, 1024] ----
            hT = []
            for at in range(2):
                kp = kps[at]
                qb = qsb[:, at * 32:(at + 1) * 32][:, :, None].to_broadcast(
                    [128, BROWS, K])
                nc.vector.tensor_tensor(
                    kp[:].rearrange("p (q k) -> p q k", k=K),
                    kp[:].rearrange("p (q k) -> p q k", k=K),
                    qb, mybir.AluOpType.add)
                h = sb.tile([128, NCD], bf16, tag=f"hT{at}", name=f"hT{at}")
                nc.scalar.activation(h[:], kp[:], mybir.ActivationFunctionType.Tanh,
                                     bias=bqm[:, at:at + 1])
                hT.append(h)

            # ---- scores candidate-major: sc[p, t] = h[:, t*128+p] . Ws ----
            for t in range(8):
                for at in range(2):
                    nc.tensor.matmul(small1[:, 64 + t:64 + t + 1],
                                     lhsT=hT[at][:, t * 128:(t + 1) * 128],
                                     rhs=Ws(at), start=(at == 0), stop=(at == 1))
            e_col = sb.tile([128, 8], f32, tag="e_col")
            nc.scalar.activation(e_col[:], small1[:, 64:72],
                                 mybir.ActivationFunctionType.Exp)

            # ---- w2[p, t, qc] = e_col[p, t] * mask[p, t*32+qc] (bf16) ----
            w2 = sb.tile([128, 8, BROWS], bf16, tag="w2")
            ebr = e_col[:, :, None].to_broadcast([128, 8, BROWS])
            nc.vector.tensor_tensor(w2[:], ebr,
                                    mask.rearrange("p (t q) -> p t q", q=BROWS),
                                    mybir.AluOpType.mult)

            # ---- den row + reciprocal + broadcast ----
            for t in range(8):
                nc.tensor.matmul(small2[0:1, 0:32], lhsT=onec, rhs=w2[:, t, :],
                                 start=(t == 0), stop=(t == 7))
            rrow = sb.tile([1, 32], bf16, tag="rrow")
            with nc.allow_low_precision(reason="softmax denom recip to bf16 for matmul broadcast"):
                nc.vector.reciprocal(rrow[:], small2[0:1, 0:32])
            nc.tensor.matmul(small2[:, 32:64], lhsT=oner, rhs=rrow[:],
                             start=True, stop=True)
            rbsb = sb.tile([128, 32], f32, tag="rbsb")
            nc.scalar.copy(out=rbsb[:], in_=small2[:, 32:64])

            # ---- attT[p, dc*32+qc] = sum_t knn[:, t, dc-block] ^T w2 ----
            for dc in range(4):
                for t in range(8):
                    nc.tensor.matmul(attps[:, dc * 32:(dc + 1) * 32],
                                     lhsT=knn[:, t, dc * 128:(dc + 1) * 128],
                                     rhs=w2[:, t, :], start=(t == 0), stop=(t == 7))
            attbf = sb.tile([128, 4, 32], bf16, tag="attbf")
            rb = rbsb[:][:, None, :].to_broadcast([128, 4, 32])
            nc.vector.tensor_tensor(attbf[:],
                                    attps[:].rearrange("p (d q) -> p d q", q=32),
                                    rb, mybir.AluOpType.mult)

            # ---- classifier ----
            for m in range(8):
                lhsT = qT(m) if m < 4 else attbf[:, m - 4, :]
                nc.tensor.matmul(small2[0:BROWS, 64:164], lhsT=lhsT, rhs=Wc(m),
                                 start=(m == 0), stop=(m == 7))
            out_sb = sb.tile([BROWS, C], f32, tag="out_sb")
            nc.scalar.copy(out=out_sb[:], in_=small2[0:BROWS, 64:164])
            nc.sync.dma_start(out=out_d[:, :], in_=out_sb[:])
    nc.finalize()
    return nc


def _phase1_nc():
    global _PH1
    if _PH1 is None:
        _PH1 = _build_phase1()
    return _PH1


def _phase2_nc():
    global _PH2
    if _PH2 is None:
        _PH2 = _build_phase2()
    return _PH2


def kernel(query_feat, memory_keys, Wq, bq, Wm, bm, Ws, bs, Wc, bc):
    query_feat = np.asarray(query_feat, np.float32)
    memory_keys = np.asarray(memory_keys, np.float32)

    # ---- host prep: normalize + fp8 quantize + shard/layout keys ----
    kn = np.sqrt((memory_keys ** 2).sum(axis=1))
    khat = memory_keys * (F8SCALE / np.maximum(kn, 1e-8))[:, None]
    k8 = np.zeros((NPAD, D), np_f8)
    k8[:N] = khat.astype(np_f8)
    # keys_d[p, slot, m, t, j] = k8[c*SHARD + CHUNK_ORDER[slot]*CHUNK + j,
    #                               m*256 + t*128 + p]
    kv = k8.reshape(NC_CORES, NCHUNK, CHUNK, 2, 2, 128)   # c ch j m t p
    kv = kv[:, CHUNK_ORDER]                               # slot reorder
    kv = kv.transpose(0, 5, 1, 3, 4, 2)                   # c p slot m t j
    kv = np.ascontiguousarray(kv)

    q32 = np.maximum(query_feat, 0)
    qn = np.sqrt((q32 ** 2).sum(axis=1))
    qhat = q32 * (F8SCALE / np.maximum(qn, 1e-8))[:, None]
    q8 = qhat.astype(np_f8)
    # qT_d[p, qt, m, t, qq] = q8[qt*128+qq, m*256+t*128+p]
    qv = q8.reshape(2, 128, 2, 2, 128).transpose(4, 0, 2, 3, 1)
    qv = np.ascontiguousarray(qv)

    ph1 = _phase1_nc()
    in_maps = [{"keys": kv[c], "qT": qv} for c in range(NC_CORES)]
    res1 = run_bass_kernel_spmd(ph1, in_maps, core_ids=list(range(NC_CORES)))

    # ---- host merge: top-8 windows per supergroup, expand, exact re-score ----
    vals = np.empty((B, NC_CORES, L3W), np.float32)
    for c in range(NC_CORES):
        v = np.asarray(res1.results[c]["l3"]).astype(np.float32)  # [2, 128, 1600]
        vals[:128, c] = v[0]
        vals[128:, c] = v[1]

    # full supers: window (c, s, i) -> keys c*SHARD + (8s+cc)*CHUNK + i
    vfull = vals[:, :, :1536].reshape(B, NC_CORES, 3, 512)
    i8 = np.argpartition(-vfull, 7, axis=3)[:, :, :, :8]          # [B, 8, 3, 8]
    base = (np.arange(NC_CORES) * SHARD)[None, :, None, None]
    sbase = (np.arange(3) * (SUPER * CHUNK))[None, None, :, None]
    kfull = (base + sbase + i8)[..., None] + (np.arange(8) * CHUNK)[None, None, None, None, :]
    # small super: window (c, i) -> keys c*SHARD + 24*CHUNK + i + 64*m
    vsm = vals[:, :, 1536:]                                       # [B, 8, 64]
    is8 = np.argpartition(-vsm, 7, axis=2)[:, :, :8]              # [B, 8, 8]
    ksm = (np.arange(NC_CORES) * SHARD)[None, :, None, None] + 24 * CHUNK \
        + is8[..., None] + (np.arange(8) * 64)[None, None, None, :]
    cand = np.concatenate([kfull.reshape(B, -1), ksm.reshape(B, -1)], axis=1)
    cand = np.sort(cand, axis=1)                                  # ties -> lowest idx

    safe = np.minimum(cand, N - 1)
    gk = memory_keys[safe]                                        # [B, 2048, 512]
    dots = np.matmul(gk, q32[:, :, None].astype(np.float32))[:, :, 0]
    cos = dots / np.maximum(qn[:, None] * kn[safe], np.float32(1e-8))
    cos[cand >= N] = -np.inf
    order = np.argsort(-cos, axis=1, kind="stable")[:, :K]
    top_idx = np.take_along_axis(safe, order, axis=1)             # [B, 32]
    knn = memory_keys[top_idx]                                    # [B, 32, 512]

    # ---- phase 2 (batch sharded, bf16) ----
    ph2 = _phase2_nc()
    PW = 3365
    panel = np.zeros((128, PW), np.float32)
    Wq_a = np.asarray(Wq, np.float32)
    Wm_a = np.asarray(Wm, np.float32)
    Ws_a = np.asarray(Ws, np.float32).reshape(A)
    Wc_a = np.asarray(Wc, np.float32)
    panel[:, 0:1024] = Wm_a.reshape(4, 128, 256).transpose(1, 0, 2).reshape(128, 1024)
    panel[:, 1024:1026] = Ws_a.reshape(2, 128).T
    msk = np.zeros((128, 8, 32), np.float32)
    p_arr = np.arange(128)
    for t in range(8):
        msk[p_arr, t, 4 * t + p_arr // 32] = 1.0
    panel[:, 1026:1282] = msk.reshape(128, 256)
    panel[:, 1282] = 1.0
    panel[0, 1283:1411] = 1.0
    panel[:, 1411:1413] = (np.asarray(bq, np.float32)
                           + np.asarray(bm, np.float32)).reshape(2, 128).T
    panel[:, 1413:2437] = Wq_a.reshape(4, 128, 256).transpose(1, 0, 2).reshape(128, 1024)
    panel[:, 2565:3365] = Wc_a.reshape(8, 128, C).transpose(1, 0, 2).reshape(128, 800)

    in_maps2 = []
    for c in range(NC_CORES):
        rows = slice(c * BROWS, (c + 1) * BROWS)
        pc = panel.copy()
        pc[:, 2437:2565] = q32[rows].T.reshape(4, 128, 32).transpose(1, 0, 2).reshape(128, 128)
        knn_c = knn[rows].reshape(NCD, D)
        knnT_c = knn_c.T.reshape(4, 128, NCD).transpose(1, 0, 2)
        knnr_c = knn_c.reshape(8, 128, D).transpose(1, 0, 2)
        in_maps2.append({
            "panel": pc.astype(np_bf16),
            "knnT": np.ascontiguousarray(knnT_c).astype(np_bf16),
            "knn": np.ascontiguousarray(knnr_c).astype(np_bf16),
        })
    res2 = run_bass_kernel_spmd(ph2, in_maps2, core_ids=list(range(NC_CORES)))
    out = np.concatenate([np.asarray(res2.results[c]["out"]) for c in range(NC_CORES)],
                         axis=0)
    return (out + np.asarray(bc, np.float32)[None, :]).astype(np.float32)


# revision 12
# speedup vs baseline: 2.6034x; 1.0087x over previous
"""Trainium2 Bass kernel for retrieval-knn attention classifier (nn_MA_51866025067137).

Strategy (8 NeuronCores):
  Phase 1 — memory_keys sharded along N (12800 keys/core, padded 100000->102400
  with zero rows).  Keys and queries are L2-normalized on host and cast to
  fp8-e4m3 (x64 scale).  Each core ranks all 256 queries against its shard with
  DoubleRow fp8 matmuls (2 k-tiles per matmul, 256-deep contraction).  The
  Activation engine evicts sim tiles from PSUM to bf16 SBUF (the DVE handles a
  couple of tiles directly via windowed reduce_max); the DVE then folds three
  levels of pairwise tensor_max (bf16 at 2x) yielding one bf16 max per 8-key
  window.  The [128, 1600] window-max arrays stream out per supergroup; the
  host picks the top-8 windows per 4096-key supergroup, expands each window to
  its 8 keys, re-scores candidates exactly in fp32, and takes the global
  top-32.
  Phase 2 — batch sharded (32 queries/core): memory-attention module
  (tanh(qWq + knnWm + b) -> scores -> softmax -> weighted sum) and classifier
  in bf16.  Scores are computed candidate-major so softmax needs no transpose
  bounce; weights stream in a fused panel; knnT streams per-dc-chunk from the
  Pool queue so the kproj matmuls start early; dummy matmuls warm the PE
  p-state during the DMA lead-in.
"""

import numpy as np
import ml_dtypes

import concourse.bacc as bacc
import concourse.mybir as mybir
from concourse.tile import TileContext
from concourse.bass_utils import run_bass_kernel_spmd

# problem dims (hardcoded per harness contract)
B, N, D = 256, 100000, 512
A, C, K = 256, 100, 32
NC_CORES = 8
NPAD = 102400             # 8 * 12800
SHARD = NPAD // NC_CORES  # 12800
CHUNK = 512
NCHUNK = SHARD // CHUNK   # 25
SUPER = 8                 # chunks per supergroup (4096 keys)
WIN = 8                   # keys per candidate window
L3W = 3 * 512 + 64        # 1600 window-maxes per qt per core
BROWS = B // NC_CORES     # 32 rows per core in phase 2
NCD = BROWS * K           # 1024 candidate rows in phase 2
F8SCALE = 64.0

# phase-1 schedule knobs (tuned against the instruction cost model)
CHUNK_ORDER = [24] + list(range(24))      # dram slot order; chunk 24 first
PIECE_PLAN = [3] + [2] * 11               # chunks per key DMA
KEY_BUFS = 6
EB_BUFS = 6
DIRECT = {(1, 0), (3, 0), (5, 0)}         # (ft, qt) tiles reduced by DVE
P2_WARMUP = 2                             # phase-2 PE warmup matmuls

f32 = mybir.dt.float32
bf16 = mybir.dt.bfloat16
f8 = mybir.dt.float8e4
DR = mybir.MatmulPerfMode.DoubleRow

np_f8 = ml_dtypes.float8_e4m3
np_bf16 = ml_dtypes.bfloat16

_PH1 = None
_PH2 = None


def _build_phase1():
    nc = bacc.Bacc("TRN2", target_bir_lowering=False)
    keys_d = nc.dram_tensor("keys", [128, NCHUNK, 2, 2, CHUNK], f8, kind="ExternalInput")
    qT_d = nc.dram_tensor("qT", [128, 2, 2, 2, 128], f8, kind="ExternalInput")
    l3_d = nc.dram_tensor("l3", [2, 128, L3W], bf16, kind="ExternalOutput")

    with TileContext(nc) as tc:
        with (
            tc.tile_pool(name="qp", bufs=1) as qpool,
            tc.tile_pool(name="keys", bufs=KEY_BUFS) as keyp,
            tc.tile_pool(name="lv", bufs=1) as lvp,
            tc.tile_pool(name="eb", bufs=EB_BUFS) as ebp,
            tc.tile_pool(name="psum", bufs=2, space="PSUM") as psump,
        ):
            qt_t = qpool.tile([128, 2, 2, 2, 128], f8, tag="q")
            nc.gpsimd.dma_start(out=qt_t[:], in_=qT_d[:, :, :, :, :])
            loaded = {}
            lo = 0
            for cnt in PIECE_PLAN:
                hi = lo + cnt
                kt = keyp.tile([128, cnt, 2, 2, CHUNK], f8, tag="kt", name="kt")
                nc.gpsimd.dma_start(out=kt[:], in_=keys_d[:, lo:hi])
                for s in range(lo, hi):
                    loaded[CHUNK_ORDER[s]] = (kt, s - lo)
                lo = hi

            sl1 = {}
            l3o = [lvp.tile([128, L3W], bf16, tag=f"l3o{qt}", name=f"l3o{qt}")
                   for qt in range(2)]

            def fold(out, in0, in1):
                nc.vector.tensor_max(out=out, in0=in0, in1=in1)

            def mm(ps, qt, chunks):
                for ci, ch in enumerate(chunks):
                    kt, lc = loaded[ch]
                    for m in range(2):
                        nc.tensor.matmul(
                            ps[:, ci * 512:(ci + 1) * 512],
                            lhsT=qt_t[:, qt, m, :, :],
                            rhs=kt[:, lc, m, :, :],
                            start=(m == 0), stop=(m == 1),
                            perf_mode=DR,
                        )

            # small supergroup (chunk 24) first: its data is in piece 0
            for qt in range(2):
                ps = psump.tile([128, 2048], f32, tag="ps", name="ps_sm")
                mm(ps, qt, [24])
                sm1 = lvp.tile([128, 256], bf16, tag=f"sm1_{qt}", name=f"sm1_{qt}")
                pv = ps[:, 0:512].rearrange("p (two x) -> p x two", two=2)
                nc.vector.reduce_max(out=sm1[:], in_=pv, axis=mybir.AxisListType.X)
                sm2 = lvp.tile([128, 128], bf16, tag=f"sm2_{qt}", name=f"sm2_{qt}")
                fold(sm2[:], sm1[:, 0:128], sm1[:, 128:256])
                fold(l3o[qt][:, 1536:1600], sm2[:, 0:64], sm2[:, 64:128])
                nc.sync.dma_start(out=l3_d[qt, :, 1536:1600],
                                  in_=l3o[qt][:, 1536:1600])

            # full psum tiles: ft covers chunks 4ft..4ft+3 ([128, 2048], 4 banks)
            for ft in range(6):
                for qt in range(2):
                    ps = psump.tile([128, 2048], f32, tag="ps", name="ps_ft")
                    mm(ps, qt, [4 * ft, 4 * ft + 1, 4 * ft + 2, 4 * ft + 3])
                    s = ft // 2
                    half = ft % 2
                    if half == 0:
                        sl1[(qt, s)] = lvp.tile([128, 2048], bf16, tag=f"sl1_{qt}",
                                                name=f"sl1_{qt}_{s}")
                    dst = sl1[(qt, s)][:, half * 1024:(half + 1) * 1024]
                    if (ft, qt) in DIRECT:
                        pv = ps[:].rearrange("p (two x) -> p x two", two=2)
                        nc.vector.reduce_max(out=dst, in_=pv,
                                             axis=mybir.AxisListType.X)
                    else:
                        eb = ebp.tile([128, 2048], bf16, tag="eb", name="eb")
                        nc.scalar.copy(out=eb[:], in_=ps[:])
                        fold(dst, eb[:, 0:1024], eb[:, 1024:2048])
                    if half == 1:
                        t1 = sl1[(qt, s)]
                        l2 = lvp.tile([128, 1024], bf16, tag=f"sl2_{qt}",
                                      name=f"sl2_{qt}_{s}")
                        fold(l2[:], t1[:, 0:1024], t1[:, 1024:2048])
                        fold(l3o[qt][:, s * 512:(s + 1) * 512],
                             l2[:, 0:512], l2[:, 512:1024])
                        nc.sync.dma_start(out=l3_d[qt, :, s * 512:(s + 1) * 512],
                                          in_=l3o[qt][:, s * 512:(s + 1) * 512])
    nc.finalize()
    return nc


def _build_phase2():
    nc = bacc.Bacc("TRN2", target_bir_lowering=False)
    # bf16 weight/constant panel shared by all cores + per-core tensors
    # panel columns (PW = 3365):
    #   Wm   [0, 1024)       [p, dc*256 + at*128 + a]
    #   Ws   [1024, 1026)    [p, at]
    #   mask [1026, 1282)    [p, t*32 + qc] = 1 if qc == 4t + p//32 else 0
    #   onec [1282]          all-ones column
    #   oner [1283, 1411)    row of ones on partition 0 only
    #   bqm  [1411, 1413)    [p, at] = (bq+bm) in bf16
    #   Wq   [1413, 2437)
    #   qT   [2437, 2565)    [p, dc*32 + q]  (relu'd query, bf16)
    #   Wc   [2565, 3365)    [p, m*100 + j]
    PW = 3365
    OWM, OWS, OMSK, OONE, OONR, OBQ, OWQ, OQT, OWC = (
        0, 1024, 1026, 1282, 1283, 1411, 1413, 2437, 2565)
    SPLIT1, SPLIT2 = 1413, 2565
    panel_d = nc.dram_tensor("panel", [128, PW], bf16, kind="ExternalInput")
    knnT_d = nc.dram_tensor("knnT", [128, 4, NCD], bf16, kind="ExternalInput")
    knn_d = nc.dram_tensor("knn", [128, 8, D], bf16, kind="ExternalInput")
    out_d = nc.dram_tensor("out", [BROWS + 1, 2 * C], f32, kind="ExternalOutput")

    with TileContext(nc) as tc:
        with (
            tc.tile_pool(name="sb", bufs=1) as sb,
            tc.tile_pool(name="ps", bufs=1, space="PSUM") as psp,
        ):
            panel = sb.tile([128, PW], bf16, tag="panel")
            knnT = sb.tile([128, 4, NCD], bf16, tag="knnT")
            knn = sb.tile([128, 8, D], bf16, tag="knn")
            # Wm + small constants first; knnT per-dc on the Pool queue;
            # Wq/qT after the knnT stream; Wc/knn late on the Pool queue
            nc.sync.dma_start(out=panel[:, 0:SPLIT1], in_=panel_d[:, 0:SPLIT1])
            for dc in range(4):
                nc.gpsimd.dma_start(out=knnT[:, dc, :], in_=knnT_d[:, dc, :])
            nc.sync.dma_start(out=panel[:, SPLIT1:SPLIT2],
                              in_=panel_d[:, SPLIT1:SPLIT2])
            nc.gpsimd.dma_start(out=knn[:], in_=knn_d[:, :, :])
            nc.gpsimd.dma_start(out=panel[:, SPLIT2:], in_=panel_d[:, SPLIT2:])

            Wq = lambda dc, at: panel[:, OWQ + dc * 256 + at * 128: OWQ + dc * 256 + (at + 1) * 128]
            Wm = lambda dc, at: panel[:, OWM + dc * 256 + at * 128: OWM + dc * 256 + (at + 1) * 128]
            Ws = lambda at: panel[:, OWS + at: OWS + at + 1]
            Wc = lambda m: panel[:, OWC + m * C: OWC + (m + 1) * C]
            mask = panel[:, OMSK:OMSK + 256]
            onec = panel[:, OONE:OONE + 1]
            bqm = panel[:, OBQ:OBQ + 2]
            qT = lambda dc: panel[:, OQT + dc * 32: OQT + (dc + 1) * 32]

            # psum small1: qp0 [0:32], qp1 [32:64], sc [64:72]
            small1 = psp.tile([128, 72], f32, tag="small1")
            # psum outp: [0:32, 0:100] q@Wcq ; [0:32, 100:200] attsum@Wca ;
            #            [32:33, 0:32] den
            outp = psp.tile([BROWS + 1, 2 * C], f32, tag="outp")
            attps = psp.tile([128, 128], f32, tag="attps")

            # PE p-state warmup on scratch data during the DMA lead-in
            wsb = sb.tile([128, 512], bf16, tag="wsb")
            nc.vector.memset(wsb[:].bitcast(mybir.dt.uint16), 0)
            wps = psp.tile([128, 512], f32, tag="wps")
            for _ in range(P2_WARMUP):
                nc.tensor.matmul(wps[:], lhsT=wsb[:, 0:128], rhs=wsb[:],
                                 start=True, stop=True, skip_group_check=True)

            # ---- kprojT chasing the knnT stream; qproj interleaved ----
            kps = [psp.tile([128, NCD], f32, tag="kp", bufs=2, name=f"kp{at}")
                   for at in range(2)]
            qsb = sb.tile([128, 64], f32, tag="qsb")
            for dc in range(4):
                for at in range(2):
                    for half in range(2):
                        nc.tensor.matmul(kps[at][:, half * 512:(half + 1) * 512],
                                         lhsT=Wm(dc, at),
                                         rhs=knnT[:, dc, half * 512:(half + 1) * 512],
                                         start=(dc == 0), stop=(dc == 3))
                if dc == 1:
                    for at in range(2):
                        for qdc in range(4):
                            nc.tensor.matmul(small1[:, at * 32:(at + 1) * 32],
                                             lhsT=Wq(qdc, at), rhs=qT(qdc),
                                             start=(qdc == 0), stop=(qdc == 3))
                    nc.scalar.copy(out=qsb[:], in_=small1[:, 0:64])
                    for m in range(4):
                        nc.tensor.matmul(outp[0:BROWS, 0:C], lhsT=qT(m), rhs=Wc(m),
                                         start=(m == 0), stop=(m == 3))

            # ---- + qproj broadcast, tanh -> hT[at] bf16 [128, 1024] ----
            hT = []
            for at in range(2):
                kp = kps[at]
                qb = qsb[:, at * 32:(at + 1) * 32][:, :, None].to_broadcast(
                    [128, BROWS, K])
                nc.vector.tensor_tensor(
                    kp[:].rearrange("p (q k) -> p q k", k=K),
                    kp[:].rearrange("p (q k) -> p q k", k=K),
                    qb, mybir.AluOpType.add)
                h = sb.tile([128, NCD], bf16, tag=f"hT{at}", name=f"hT{at}")
                nc.scalar.activation(h[:], kp[:], mybir.ActivationFunctionType.Tanh,
                                     bias=bqm[:, at:at + 1])
                hT.append(h)

            # ---- scores candidate-major: sc[p, t] = h[:, t*128+p] . Ws ----
            for t in range(8):
                for at in range(2):
                    nc.tensor.matmul(small1[:, 64 + t:64 + t + 1],
                                     lhsT=hT[at][:, t * 128:(t + 1) * 128],
                                     rhs=Ws(at), start=(at == 0), stop=(at == 1))
            e_col = sb.tile([128, 8], f32, tag="e_col")
            nc.scalar.activation(e_col[:], small1[:, 64:72],
                                 mybir.ActivationFunctionType.Exp)

            # ---- w2[p, t, qc] = e_col[p, t] * mask[p, t*32+qc] (bf16) ----
            w2 = sb.tile([128, 8, BROWS], bf16, tag="w2")
            ebr = e_col[:, :, None].to_broadcast([128, 8, BROWS])
            nc.vector.tensor_tensor(w2[:], ebr,
                                    mask.rearrange("p (t q) -> p t q", q=BROWS),
                                    mybir.AluOpType.mult)

            # ---- den rows (normalization deferred to host) ----
            for t in range(8):
                nc.tensor.matmul(outp[BROWS:BROWS + 1, 0:32], lhsT=onec,
                                 rhs=w2[:, t, :], start=(t == 0), stop=(t == 7))

            # ---- attsumT[p, dc*32+qc] = sum_t knn[:, t, dc-block] ^T w2 ----
            for dc in range(4):
                for t in range(8):
                    nc.tensor.matmul(attps[:, dc * 32:(dc + 1) * 32],
                                     lhsT=knn[:, t, dc * 128:(dc + 1) * 128],
                                     rhs=w2[:, t, :], start=(t == 0), stop=(t == 7))
            attbf = sb.tile([128, 4, 32], bf16, tag="attbf")
            nc.scalar.copy(out=attbf[:],
                           in_=attps[:].rearrange("p (d q) -> p d q", q=32))
            # ---- att-half of classifier (unnormalized) ----
            for m in range(4):
                nc.tensor.matmul(outp[0:BROWS, C:2 * C], lhsT=attbf[:, m, :],
                                 rhs=Wc(m + 4), start=(m == 0), stop=(m == 3))
            out_sb = sb.tile([BROWS + 1, 2 * C], f32, tag="out_sb")
            nc.scalar.copy(out=out_sb[:], in_=outp[:])
            nc.sync.dma_start(out=out_d[:, :], in_=out_sb[:])
    nc.finalize()
    return nc


def _phase1_nc():
    global _PH1
    if _PH1 is None:
        _PH1 = _build_phase1()
    return _PH1


def _phase2_nc():
    global _PH2
    if _PH2 is None:
        _PH2 = _build_phase2()
    return _PH2


def kernel(query_feat, memory_keys, Wq, bq, Wm, bm, Ws, bs, Wc, bc):
    query_feat = np.asarray(query_feat, np.float32)
    memory_keys = np.asarray(memory_keys, np.float32)

    # ---- host prep: normalize + fp8 quantize + shard/layout keys ----
    kn = np.sqrt((memory_keys ** 2).sum(axis=1))
    khat = memory_keys * (F8SCALE / np.maximum(kn, 1e-8))[:, None]
    k8 = np.zeros((NPAD, D), np_f8)
    k8[:N] = khat.astype(np_f8)
    # keys_d[p, slot, m, t, j] = k8[c*SHARD + CHUNK_ORDER[slot]*CHUNK + j,
    #                               m*256 + t*128 + p]
    kv = k8.reshape(NC_CORES, NCHUNK, CHUNK, 2, 2, 128)   # c ch j m t p
    kv = kv[:, CHUNK_ORDER]                               # slot reorder
    kv = kv.transpose(0, 5, 1, 3, 4, 2)                   # c p slot m t j
    kv = np.ascontiguousarray(kv)

    q32 = np.maximum(query_feat, 0)
    qn = np.sqrt((q32 ** 2).sum(axis=1))
    qhat = q32 * (F8SCALE / np.maximum(qn, 1e-8))[:, None]
    q8 = qhat.astype(np_f8)
    # qT_d[p, qt, m, t, qq] = q8[qt*128+qq, m*256+t*128+p]
    qv = q8.reshape(2, 128, 2, 2, 128).transpose(4, 0, 2, 3, 1)
    qv = np.ascontiguousarray(qv)

    ph1 = _phase1_nc()
    in_maps = [{"keys": kv[c], "qT": qv} for c in range(NC_CORES)]
    res1 = run_bass_kernel_spmd(ph1, in_maps, core_ids=list(range(NC_CORES)))

    # ---- host merge: top-8 windows per supergroup, expand, exact re-score ----
    vals = np.empty((B, NC_CORES, L3W), np.float32)
    for c in range(NC_CORES):
        v = np.asarray(res1.results[c]["l3"]).astype(np.float32)  # [2, 128, 1600]
        vals[:128, c] = v[0]
        vals[128:, c] = v[1]

    # full supers: window (c, s, i) -> keys c*SHARD + (8s+cc)*CHUNK + i
    vfull = vals[:, :, :1536].reshape(B, NC_CORES, 3, 512)
    i8 = np.argpartition(-vfull, 7, axis=3)[:, :, :, :8]          # [B, 8, 3, 8]
    base = (np.arange(NC_CORES) * SHARD)[None, :, None, None]
    sbase = (np.arange(3) * (SUPER * CHUNK))[None, None, :, None]
    kfull = (base + sbase + i8)[..., None] + (np.arange(8) * CHUNK)[None, None, None, None, :]
    # small super: window (c, i) -> keys c*SHARD + 24*CHUNK + i + 64*m
    vsm = vals[:, :, 1536:]                                       # [B, 8, 64]
    is8 = np.argpartition(-vsm, 7, axis=2)[:, :, :8]              # [B, 8, 8]
    ksm = (np.arange(NC_CORES) * SHARD)[None, :, None, None] + 24 * CHUNK \
        + is8[..., None] + (np.arange(8) * 64)[None, None, None, :]
    cand = np.concatenate([kfull.reshape(B, -1), ksm.reshape(B, -1)], axis=1)
    cand = np.sort(cand, axis=1)                                  # ties -> lowest idx

    safe = np.minimum(cand, N - 1)
    gk = memory_keys[safe]                                        # [B, 2048, 512]
    dots = np.matmul(gk, q32[:, :, None].astype(np.float32))[:, :, 0]
    cos = dots / np.maximum(qn[:, None] * kn[safe], np.float32(1e-8))
    cos[cand >= N] = -np.inf
    order = np.argsort(-cos, axis=1, kind="stable")[:, :K]
    top_idx = np.take_along_axis(safe, order, axis=1)             # [B, 32]
    knn = memory_keys[top_idx]                                    # [B, 32, 512]

    # ---- phase 2 (batch sharded, bf16) ----
    ph2 = _phase2_nc()
    PW = 3365
    panel = np.zeros((128, PW), np.float32)
    Wq_a = np.asarray(Wq, np.float32)
    Wm_a = np.asarray(Wm, np.float32)
    Ws_a = np.asarray(Ws, np.float32).reshape(A)
    Wc_a = np.asarray(Wc, np.float32)
    panel[:, 0:1024] = Wm_a.reshape(4, 128, 256).transpose(1, 0, 2).reshape(128, 1024)
    panel[:, 1024:1026] = Ws_a.reshape(2, 128).T
    msk = np.zeros((128, 8, 32), np.float32)
    p_arr = np.arange(128)
    for t in range(8):
        msk[p_arr, t, 4 * t + p_arr // 32] = 1.0
    panel[:, 1026:1282] = msk.reshape(128, 256)
    panel[:, 1282] = 1.0
    panel[0, 1283:1411] = 1.0
    panel[:, 1411:1413] = (np.asarray(bq, np.float32)
                           + np.asarray(bm, np.float32)).reshape(2, 128).T
    panel[:, 1413:2437] = Wq_a.reshape(4, 128, 256).transpose(1, 0, 2).reshape(128, 1024)
    panel[:, 2565:3365] = Wc_a.reshape(8, 128, C).transpose(1, 0, 2).reshape(128, 800)

    in_maps2 = []
    for c in range(NC_CORES):
        rows = slice(c * BROWS, (c + 1) * BROWS)
        pc = panel.copy()
        pc[:, 2437:2565] = q32[rows].T.reshape(4, 128, 32).transpose(1, 0, 2).reshape(128, 128)
        knn_c = knn[rows].reshape(NCD, D)
        knnT_c = knn_c.T.reshape(4, 128, NCD).transpose(1, 0, 2)
        knnr_c = knn_c.reshape(8, 128, D).transpose(1, 0, 2)
        in_maps2.append({
            "panel": pc.astype(np_bf16),
            "knnT": np.ascontiguousarray(knnT_c).astype(np_bf16),
            "knn": np.ascontiguousarray(knnr_c).astype(np_bf16),
        })
    res2 = run_bass_kernel_spmd(ph2, in_maps2, core_ids=list(range(NC_CORES)))
    outs = []
    for c in range(NC_CORES):
        o = np.asarray(res2.results[c]["out"])        # [33, 200]
        den = o[BROWS, 0:BROWS]
        outs.append(o[:BROWS, :C] + o[:BROWS, C:] / den[:, None])
    out = np.concatenate(outs, axis=0)
    return (out + np.asarray(bc, np.float32)[None, :]).astype(np.float32)
